# revision 3
# baseline (speedup 1.0000x reference)
"""GateRetention Trainium2 kernel (Bass/Tile), 8-core tensor-parallel.

Sharding: core grid (batch b = core//4, head-group g = core%4); each core owns
4 heads (512 cols of the q/k/v/g projections, 512 rows of Wo) of one batch.
RMS-norm statistics are AllReduced across each batch's 4 cores; gate logits
are contraction-sharded (host pre-adds x+c, each core contracts a 512-row
quarter for all 16 heads) and ReduceScattered so each core gets its 4 heads.
Out-proj partials are summed on the host (row-parallel TP gather).

Precision: all projections and retention in fp16 with fp32 PSUM accumulation;
a 2^-2 exponent shift on vfac keeps decayed v tiles in fp16 range.  The
rowfac (per-token decay * q-norm * scale) is folded into the subln norm
scale f = rf * rsqrt(rf^2 * sumsq/HD + eps), so the raw retention output is
normalized+scaled in one pass (exactly equal to norm(rf*o_raw)).

kernel(**inputs) takes the FULL inputs from reference.setup_inputs() and
returns the FULL [B, T, DIM] fp32 output.
"""
import os
import sys

sys.path.insert(0, "/opt/trn_rl_repo")

import numpy as np

import concourse.bass as bass
import concourse.bacc as bacc
import concourse.tile as tile
import concourse.mybir as mybir
from concourse import bass_utils

F32 = mybir.dt.float32
F32R = mybir.dt.float32r
F16 = mybir.dt.float16
AX = mybir.AxisListType
ALU = mybir.AluOpType
ACTF = mybir.ActivationFunctionType

B, T, DIM = 2, 4096, 2048
H, HD = 16, 128
CS = 256
NCH = T // CS              # 16 chunks
EPS = 1e-5
GLN = 16.0
SCALE = HD ** -0.5
NCORE = 8
HPC = 4                    # heads per core
PCOLS = HPC * HD           # 512 cols per core
NBLK = T // 128            # 32 token blocks of 128
NT = T // 512              # 8 token n-tiles
VSH = 2.0 ** -2            # fp16 range shift on vv; inverse folded into rowfac

DEBUG_LVL = int(os.environ.get("GR_DEBUG", "0"))
DEBUG = bool(DEBUG_LVL)
TRACE = bool(int(os.environ.get("GR_TRACE", "0")))
PET = bool(int(os.environ.get("GR_PET", "0")))      # PE-transpose fallback for o_n

_cache = {}


def _consts_np():
    """[128, 904] fp32: identity | Lm | Om | Um | ones8 | Lc | M0."""
    ident = np.eye(128, dtype=np.float32)
    jj, ii = np.meshgrid(np.arange(128), np.arange(128), indexing="ij")
    Lm = np.where(jj <= ii, -1.0 / GLN, 0.0).astype(np.float32)
    Om = np.full((128, 128), -1.0 / GLN, np.float32)
    Um = np.where(jj <= ii, 1.0, 0.0).astype(np.float32)
    ones = np.ones((128, 8), np.float32)
    # Lc: b_i - b_mid for block0 = +1/GLN * sum_{j>i} sp_j
    Lc = np.where(jj > ii, 1.0 / GLN, 0.0).astype(np.float32)
    M0 = np.concatenate([Um, np.ones((128, 128), np.float32)], axis=1)
    return np.concatenate([ident, Lm, Om, Um, ones, Lc, M0], axis=1)


def build(debug=False):
    nc = bacc.Bacc("TRN2", target_bir_lowering=False, debug=False,
                   enable_asserts=False, num_devices=NCORE)

    # ---------------- I/O ----------------
    xT = nc.dram_tensor("xT", [DIM, T], F16, kind="ExternalInput").ap()
    # (x+c)^T row-quarter for this core's contraction shard of the gate logits
    xcT = nc.dram_tensor("xcT", [PCOLS, T], F16, kind="ExternalInput").ap()
    wq = nc.dram_tensor("wq", [DIM, PCOLS], F16, kind="ExternalInput").ap()
    wk = nc.dram_tensor("wk", [DIM, PCOLS], F16, kind="ExternalInput").ap()
    wv = nc.dram_tensor("wv", [DIM, PCOLS], F16, kind="ExternalInput").ap()
    wg = nc.dram_tensor("wg", [DIM, PCOLS], F16, kind="ExternalInput").ap()
    wgt = nc.dram_tensor("wgt", [PCOLS, H], F16, kind="ExternalInput").ap()
    wo = nc.dram_tensor("wo", [PCOLS, DIM], F16, kind="ExternalInput").ap()
    consts = nc.dram_tensor("consts", [128, 904], F32R, kind="ExternalInput").ap()
    c16 = nc.dram_tensor("c16", [128, 129], F16, kind="ExternalInput").ap()
    out = nc.dram_tensor("out", [T, DIM], F16, kind="ExternalOutput").ap()

    def dbg(name, shape, dtype=F32):
        return nc.dram_tensor(name, shape, dtype, kind="ExternalOutput").ap()

    with tile.TileContext(nc) as tc:
        with (
            tc.tile_pool(name="const", bufs=1) as cpool,
            tc.tile_pool(name="wts", bufs=1) as wpool,
            tc.tile_pool(name="xstream", bufs=4) as xpool,
            tc.tile_pool(name="cstream", bufs=2) as ctpool,
            tc.tile_pool(name="evac", bufs=4) as epool,
            tc.tile_pool(name="persist", bufs=1) as ppool,
            tc.tile_pool(name="small", bufs=2) as spool,
            tc.tile_pool(name="ret", bufs=2) as rpool,
            tc.tile_pool(name="ps", bufs=1, space="PSUM") as psp,
            tc.tile_pool(name="dram", bufs=1, space="DRAM") as dpool,
        ):
            def ps_big():
                return psp.tile([128, 512], F32, tag="big", bufs=2, name="psbig")

            def ps_small(shape=None, dtype=F32):
                return psp.tile(shape or [128, 256], dtype, tag="small", bufs=2,
                                name="pssmall")

            def ps_oraw():
                return psp.tile([128, 2 * HPC, HD], F32, tag="oraw", bufs=2,
                                name="psoraw")

            # ---------------- constants ----------------
            cst = cpool.tile([128, 904], F32R, tag="consts")
            nc.sync.dma_start(cst[:], consts)
            ident32 = cst[:, 0:128].bitcast(F32)
            Lm = cst[:, 128:256]
            Om = cst[:, 256:384]
            Um32 = cst[:, 384:512].bitcast(F32)
            Lc = cst[:, 520:648]
            M0 = cst[:, 648:904].bitcast(F32)
            aux16 = cpool.tile([128, 129], F16, tag="i16")
            nc.sync.dma_start(aux16[:], c16)
            i16 = aux16[:, 0:128]
            ones16 = aux16[:, 128:129]

            # ---------------- DRAM scratch (fp16) ----------------
            if debug:
                qT_s = dbg("dbg_qT", [PCOLS, T], F16)
                kT_s = dbg("dbg_kT", [PCOLS, T], F16)
                vN_s = dbg("dbg_vN", [T, PCOLS], F16)
                gT_s = dbg("dbg_gT", [PCOLS, T], F16)
            else:
                qT_s = nc.dram_tensor("qT_s", [PCOLS, T], F16,
                                      kind="Internal").ap()
                kT_s = nc.dram_tensor("kT_s", [PCOLS, T], F16,
                                      kind="Internal").ap()
                vN_s = nc.dram_tensor("vN_s", [T, PCOLS], F16,
                                      kind="Internal").ap()
                gT_s = nc.dram_tensor("gT_s", [PCOLS, T], F16,
                                      kind="Internal").ap()
            ss_in = nc.dram_tensor("ss_in", [3, T], F32, kind="Internal").ap()
            ss_out = nc.dram_tensor("ss_out", [3, T], F32, kind="Internal").ap()
            gt_in = nc.dram_tensor("gt_in", [H, T], F32, kind="Internal").ap()
            gt_out = nc.dram_tensor("gt_out", [HPC, T], F32,
                                    kind="Internal").ap()

            # =========================================================
            # P1: single fused pass: q,k,g (T-layout), v (natural),
            # gate-logit partials; all fp16 math, fp32 PSUM.
            # =========================================================
            vss = ppool.tile([128, NBLK], F32, tag="vss")

            def load_w(wdram, tag):
                wt = wpool.tile([128, 16, 512], F16, tag=tag)
                nc.sync.dma_start(
                    wt[:], wdram.rearrange("(kt p) m -> p kt m", p=128))
                return wt

            wq_sb = load_w(wq, "w0")
            wk_sb = load_w(wk, "w1")
            wv_sb = load_w(wv, "w2")
            wg_sb = load_w(wg, "w3")
            wgt_sb = wpool.tile([128, HPC, H], F16, tag="wgt")
            nc.sync.dma_start(wgt_sb[:],
                              wgt.rearrange("(kt p) m -> p kt m", p=128))

            def tproj_mms(ps, wt, xth, m):
                for k in range(16):
                    nc.tensor.matmul(
                        ps[:], wt[:, k, m * 128:(m + 1) * 128],
                        xth[k // 8][:, k % 8, :], start=(k == 0), stop=(k == 15))

            for n in range(NT):
                tok = slice(n * 512, (n + 1) * 512)
                xth = []
                for h2 in range(2):
                    xt = xpool.tile([128, 8, 512], F16, tag="xt")
                    nc.sync.dma_start(
                        xt[:], xT[h2 * 1024:(h2 + 1) * 1024, tok].rearrange(
                            "(kt p) m -> p kt m", p=128))
                    xth.append(xt)
                # ---- q, k: T-layout + per-token sumsq over this core's cols
                for pi, (wsb, sdram) in enumerate(((wq_sb, qT_s), (wk_sb, kT_s))):
                    for m in range(4):
                        ps = ps_big()
                        tproj_mms(ps, wsb, xth, m)
                        ev = epool.tile([128, 512], F16, tag="ev")
                        sqt = epool.tile([128, 512], F16, tag="sq")
                        if m % 2 == 0:
                            nc.vector.tensor_copy(ev[:], ps[:])
                            nc.scalar.activation(sqt[:], ps[:], ACTF.Square)
                            nc.sync.dma_start(
                                sdram[m * 128:(m + 1) * 128, tok], ev[:])
                        else:
                            nc.scalar.copy(ev[:], ps[:])
                            nc.vector.tensor_tensor(sqt[:], ev[:], ev[:],
                                                    ALU.mult)
                            nc.scalar.dma_start(
                                sdram[m * 128:(m + 1) * 128, tok], ev[:])
                        if m == 0:
                            ssps = ps_small([1, 512])
                        nc.tensor.matmul(ssps[:1, :], ones16, sqt[:],
                                         start=(m == 0), stop=(m == 3))
                        if m == 3:
                            ssev = spool.tile([1, 512], F32, tag="ssev", bufs=2)
                            nc.vector.tensor_copy(ssev[:], ssps[:1, :])
                            nc.sync.dma_start(ss_in[pi:pi + 1, tok], ssev[:])
                # ---- g: silu, T-layout
                for m in range(4):
                    ps = ps_big()
                    tproj_mms(ps, wg_sb, xth, m)
                    ev = epool.tile([128, 512], F16, tag="ev")
                    nc.scalar.activation(ev[:], ps[:], ACTF.Silu)
                    nc.scalar.dma_start(gT_s[m * 128:(m + 1) * 128, tok], ev[:])
                # ---- v: natural layout + accumulated sumsq
                for mt in range(4):
                    msl = slice(mt * 128, (mt + 1) * 128)
                    ps = ps_big()
                    for k in range(16):
                        nc.tensor.matmul(
                            ps[:], xth[k // 8][:, k % 8, msl], wv_sb[:, k, :],
                            start=(k == 0), stop=(k == 15))
                    ev = epool.tile([128, 512], F16, tag="ev")
                    sqt = epool.tile([128, 512], F16, tag="sq")
                    nc.vector.tensor_copy(ev[:], ps[:])
                    nc.scalar.activation(
                        sqt[:], ps[:], ACTF.Square,
                        accum_out=vss[:, n * 4 + mt:n * 4 + mt + 1])
                    nc.sync.dma_start(
                        vN_s[n * 512 + mt * 128:n * 512 + (mt + 1) * 128, :],
                        ev[:])
                # ---- gate logits: contraction-sharded over (x+c) quarter
                ct = ctpool.tile([128, HPC, 512], F16, tag="ct")
                nc.sync.dma_start(
                    ct[:], xcT[:, tok].rearrange("(kt p) m -> p kt m", p=128))
                gtps = ps_small([H, 512])
                for kk in range(HPC):
                    nc.tensor.matmul(gtps[:H, :], wgt_sb[:, kk, :],
                                     ct[:, kk, :], start=(kk == 0),
                                     stop=(kk == HPC - 1))
                gstg = spool.tile([H, 512], F32, tag="gstg", bufs=2)
                nc.vector.tensor_copy(gstg[:], gtps[:H, :])
                nc.scalar.dma_start(gt_in[:, tok], gstg[:])

            # v sumsq: transpose [128, 32] -> [32, 128] -> ss_in row 2
            vssT = ps_small([128, 128])
            nc.tensor.matmul(vssT[:32, :], vss[:], ident32, is_transpose=True)
            vssev = spool.tile([32, 128], F32, tag="vssev", bufs=1)
            nc.vector.tensor_copy(vssev[:], vssT[:32, :])
            nc.sync.dma_start(
                ss_in[2:3, :].rearrange("a (b c) -> (a b) c", c=128), vssev[:])

            # =========================================================
            # P2: collectives; scales; gate decays
            # =========================================================
            nc.gpsimd.collective_compute(
                "AllReduce", ALU.add,
                replica_groups=[[0, 1, 2, 3], [4, 5, 6, 7]],
                ins=[ss_in.opt()], outs=[ss_out.opt()],
            )
            nc.gpsimd.collective_compute(
                "ReduceScatter", ALU.add,
                replica_groups=[[0, 1, 2, 3], [4, 5, 6, 7]],
                ins=[gt_in.opt()], outs=[gt_out.opt()],
            )
            # transpose [7, T] (3 sumsq + 4 gate-logit rows) into token-major
            ssgt = ppool.tile([128, NBLK, 7], F32, tag="ssgt")
            for nn_ in range(NT):
                tok = slice(nn_ * 512, (nn_ + 1) * 512)
                srt = spool.tile([7, 512], F32, tag="srt", bufs=2)
                nc.sync.dma_start(srt[0:3, :], ss_out[:, tok])
                nc.sync.dma_start(srt[3:7, :], gt_out[:, tok])
                for j in range(4):
                    tp = ps_small([128, 8])
                    nc.tensor.matmul(tp[:, :7], srt[:, j * 128:(j + 1) * 128],
                                     ident32[:7, :7], is_transpose=True)
                    nc.vector.tensor_copy(ssgt[:, nn_ * 4 + j, :], tp[:, :7])
            rsn = ppool.tile([128, NBLK, 3], F32, tag="rsn")
            nc.vector.tensor_scalar(rsn[:], ssgt[:, :, 0:3], 1.0 / DIM, EPS,
                                    ALU.mult, ALU.add)
            nc.scalar.activation(rsn[:], rsn[:], ACTF.Ln)
            nc.scalar.activation(rsn[:], rsn[:], ACTF.Exp, scale=-0.5)
            skv = ppool.tile([128, NBLK], F32, tag="skv")
            nc.vector.tensor_mul(skv[:], rsn[:, :, 1], rsn[:, :, 2])
            if debug and DEBUG_LVL >= 2:
                nc.sync.dma_start(dbg("dbg_rsn", [128, NBLK * 3]),
                                  rsn[:].rearrange("p a b -> p (a b)"))

            # gate decays: sp = softplus(-z) = ln(1 + exp(-z)); -1/GLN in Lm/Om
            gtd = ppool.tile([128, NBLK, HPC], F32R, tag="gtd")
            gtn = ppool.tile([128, NBLK, HPC], F32, tag="gtn")
            nc.scalar.activation(gtn[:], ssgt[:, :, 3:7], ACTF.Exp, scale=-1.0)
            nc.scalar.activation(gtd[:], gtn[:], ACTF.Ln, bias=1.0)

            # per chunk: recentered b' = b - b_mid via triangular matmuls;
            # eS = exp(mid-to-mid decay) for the state recurrence
            rf = ppool.tile([128, NCH, 2, HPC], F32, tag="rf")      # rowfac
            vf = ppool.tile([128, NCH, 2, HPC], F32, tag="vf")      # vfac
            eS = ppool.tile([128, NCH, HPC], F32, tag="eS")
            for ch in range(NCH):
                b0, b1 = 2 * ch, 2 * ch + 1
                p0 = ps_small([128, HPC])
                nc.tensor.matmul(p0[:], Lc, gtd[:, b0, :], start=True, stop=True)
                p1 = ps_small([128, HPC])
                nc.tensor.matmul(p1[:], Lm, gtd[:, b1, :], start=True, stop=True)
                if ch < NCH - 1:
                    pt = ps_small([128, HPC])
                    nc.tensor.matmul(pt[:], Om, gtd[:, b1, :],
                                     start=True, stop=False)
                    nc.tensor.matmul(pt[:], Om, gtd[:, b1 + 1, :],
                                     start=False, stop=True)
                    nc.scalar.activation(eS[:, ch, :], pt[:], ACTF.Exp)
                for blk01, bps in ((0, p0), (1, p1)):
                    blk = 2 * ch + blk01
                    # rowfac = exp(b') * sq * scale / VSH
                    nc.scalar.activation(rf[:, ch, blk01, :], bps[:], ACTF.Exp)
                    nc.vector.tensor_scalar(
                        rf[:, ch, blk01, :], rf[:, ch, blk01, :],
                        rsn[:, blk, 0:1], SCALE / VSH, ALU.mult, ALU.mult)
                    # vfac = exp(-b') * sk * sv * VSH
                    nc.scalar.activation(vf[:, ch, blk01, :], bps[:], ACTF.Exp,
                                         scale=-1.0)
                    nc.vector.tensor_scalar(
                        vf[:, ch, blk01, :], vf[:, ch, blk01, :],
                        skv[:, blk:blk + 1], VSH, ALU.mult, ALU.mult)
            # rf2 = rf^2 / HD (for the fused norm scale)
            rf2 = ppool.tile([128, NCH, 2, HPC], F32, tag="rf2")
            nc.vector.scalar_tensor_tensor(
                rf2[:], rf[:], 1.0 / HD, rf[:], op0=ALU.mult, op1=ALU.mult)

            if debug and DEBUG_LVL >= 3:
                nc.sync.dma_start(
                    dbg("dbg_rf", [128, NCH * 2 * HPC]),
                    rf[:].rearrange("p a b c -> p (a b c)"))
                nc.sync.dma_start(
                    dbg("dbg_vf", [128, NCH * 2 * HPC]),
                    vf[:].rearrange("p a b c -> p (a b c)"))
                nc.sync.dma_start(
                    dbg("dbg_eS", [128, NCH * HPC]),
                    eS[:].rearrange("p a b -> p (a b)"))
            if debug and DEBUG_LVL >= 2:
                nc.sync.dma_start(
                    dbg("dbg_gtd", [128, NBLK * HPC]),
                    gtd[:].bitcast(F32).rearrange("p a b -> p (a b)"))

            # =========================================================
            # P3: retention + fused norm/gate + out-proj, per chunk
            # =========================================================
            if int(os.environ.get("GR_BARRIER", "0")):
                tc.prologue_barrier()
            wo_sb = wpool.tile([128, HPC, DIM], F16, tag="wo")
            nc.sync.dma_start(wo_sb[:], wo.rearrange("(h p) m -> p h m", p=128))

            S_prev = None
            for ch in range(NCH):
                tok = slice(ch * CS, (ch + 1) * CS)
                qc = rpool.tile([128, HPC, CS], F16, tag="qc")
                kc = rpool.tile([128, HPC, CS], F16, tag="kc")
                for t_, s_ in ((qc, qT_s), (kc, kT_s)):
                    nc.sync.dma_start(
                        t_[:], s_[:, tok].rearrange("(h p) m -> p h m", p=128))
                # k natural layout via XBAR transpose DMAs (idx = hl*2 + ci)
                kn = []
                if ch < NCH - 1:
                    for hl in range(HPC):
                        for ci in range(2):
                            knt = rpool.tile([128, 128], F16,
                                             tag=f"kn{hl * 2 + ci}")
                            nc.scalar.dma_start_transpose(
                                knt[:],
                                kT_s[hl * 128:(hl + 1) * 128,
                                     ch * CS + ci * 128:ch * CS + ci * 128 + 128])
                            kn.append(knt)
                vcn = []
                for ci in range(2):
                    bt = slice(ch * CS + ci * 128, ch * CS + ci * 128 + 128)
                    vt = rpool.tile([128, PCOLS], F16, tag="vcn", bufs=4)
                    nc.sync.dma_start(vt[:], vN_s[bt, :])
                    vcn.append(vt)
                sg = rpool.tile([128, HPC, CS], F16, tag="sg")
                nc.sync.dma_start(
                    sg[:], gT_s[:, tok].rearrange("(h p) m -> p h m", p=128))
                # vv = v * vfac (gpsimd; SBUF only)
                vvt = rpool.tile([128, 2, HPC, HD], F16, tag="vvt")
                for ci in range(2):
                    for hl in range(HPC):
                        nc.gpsimd.tensor_scalar(
                            vvt[:, ci, hl, :],
                            vcn[ci][:, hl * 128:(hl + 1) * 128],
                            vf[:, ch, ci, hl:hl + 1], None, ALU.mult)
                # AT (masked): rows tj, cols ti
                at0s, at1s = [], []
                for hl in range(HPC):
                    at0ps = ps_small([128, 256])
                    nc.tensor.matmul(at0ps[:], kc[:, hl, 0:128], qc[:, hl, :],
                                     start=True, stop=True)
                    at0 = rpool.tile([128, CS], F16, tag="at0", bufs=4)
                    nc.vector.scalar_tensor_tensor(
                        at0[:], at0ps[:], 1.0, M0, op0=ALU.mult, op1=ALU.mult)
                    at0s.append(at0)
                    at1ps = ps_small([128, 128])
                    nc.tensor.matmul(at1ps[:], kc[:, hl, 128:256],
                                     qc[:, hl, 128:256], start=True, stop=True)
                    at1 = rpool.tile([128, 128], F16, tag="at1s", bufs=4)
                    nc.vector.scalar_tensor_tensor(
                        at1[:], at1ps[:], 1.0, Um32, op0=ALU.mult, op1=ALU.mult)
                    at1s.append(at1)
                # o_raw = intra + inter, regions (ci*HPC + hl)
                orps = ps_oraw()
                for hl in range(HPC):
                    for ci in range(2):
                        reg = orps[:, ci * HPC + hl, :]
                        mms = [(at0s[hl][:, ci * 128:ci * 128 + 128],
                                vvt[:, 0, hl, :])]
                        if ci == 1:
                            mms.append((at1s[hl][:], vvt[:, 1, hl, :]))
                        if ch > 0:
                            mms.append((qc[:, hl, ci * 128:ci * 128 + 128],
                                        S_prev[:, hl, :]))
                        for i, (lh, rh) in enumerate(mms):
                            nc.tensor.matmul(reg, lh, rh, start=(i == 0),
                                             stop=(i == len(mms) - 1))
                # state update: S_cur = (S_prev + kn^T vv) * eS
                if ch < NCH - 1:
                    sps = ps_small([128, HPC, HD])
                    for hl in range(HPC):
                        nc.tensor.matmul(sps[:, hl, :], kn[hl * 2][:],
                                         vvt[:, 0, hl, :], start=True,
                                         stop=False)
                        nc.tensor.matmul(sps[:, hl, :], kn[hl * 2 + 1][:],
                                         vvt[:, 1, hl, :], start=False,
                                         stop=True)
                    eSb = eS[:, ch, :].unsqueeze(2).to_broadcast(
                        [128, HPC, HD])
                    S_cur = rpool.tile([128, HPC, HD], F16, tag="S")
                    if ch > 0:
                        stmp = rpool.tile([128, HPC, HD], F32, tag="stmp")
                        nc.vector.tensor_tensor(stmp[:], sps[:], S_prev[:],
                                                ALU.add)
                        nc.vector.tensor_tensor(S_cur[:], stmp[:], eSb,
                                                ALU.mult)
                    else:
                        nc.vector.tensor_tensor(S_cur[:], sps[:], eSb,
                                                ALU.mult)
                    S_prev = S_cur
                # fused subln norm + rowfac: f = rf*rsqrt(rf^2*ss/HD + eps)
                ssum = rpool.tile([128, 2 * HPC], F32, tag="ssum")
                for idx in range(2 * HPC):
                    osq = rpool.tile([128, HD], F32, tag="osq", bufs=4)
                    nc.scalar.activation(osq[:], orps[:, idx, :], ACTF.Square,
                                         accum_out=ssum[:, idx:idx + 1])
                rfv = rf[:, ch].rearrange("p a b -> p (a b)")
                rf2v = rf2[:, ch].rearrange("p a b -> p (a b)")
                dd = rpool.tile([128, 2 * HPC], F32, tag="dd")
                nc.vector.tensor_tensor(dd[:], rf2v, ssum[:], ALU.mult)
                nc.vector.tensor_scalar(dd[:], dd[:], EPS, None, ALU.add)
                nc.scalar.activation(dd[:], dd[:], ACTF.Ln)
                nc.scalar.activation(dd[:], dd[:], ACTF.Exp, scale=-0.5)
                ff = rpool.tile([128, 2 * HPC], F32, tag="ff")
                nc.vector.tensor_tensor(ff[:], rfv, dd[:], ALU.mult)
                o_n = rpool.tile([128, 2 * HPC, HD], F16, tag="o_n")
                for half in range(2):
                    hsl = slice(half * HPC, (half + 1) * HPC)
                    nc.vector.tensor_tensor(
                        o_n[:, hsl, :], orps[:, hsl, :],
                        ff[:, hsl].unsqueeze(2).to_broadcast([128, HPC, HD]),
                        ALU.mult)
                # transpose to [chan, tok] + gate (idx = ci*HPC + hl)
                go = []
                for ci in range(2):
                    for hl in range(HPC):
                        idx = ci * HPC + hl
                        got = rpool.tile([128, 128], F16, tag=f"go{idx}")
                        if PET:
                            trp = ps_small([128, 128], F16)
                            nc.tensor.transpose(trp[:], o_n[:, idx, :], i16)
                            nc.vector.tensor_mul(
                                got[:], trp[:],
                                sg[:, hl, ci * 128:ci * 128 + 128])
                        else:
                            tro = rpool.tile([128, 128], F16, tag=f"tr{idx}")
                            nc.sync.dma_start_transpose(
                                tro[:], o_n[:, idx, :])
                            nc.gpsimd.tensor_tensor(
                                got[:], tro[:],
                                sg[:, hl, ci * 128:ci * 128 + 128], ALU.mult)
                        go.append(got)
                # out-proj for this chunk's two token tiles
                for m01 in range(2):
                    for nb in range(DIM // 512):
                        ps = ps_big()
                        nsl = slice(nb * 512, (nb + 1) * 512)
                        for hl in range(HPC):
                            nc.tensor.matmul(ps[:], go[m01 * HPC + hl][:],
                                             wo_sb[:, hl, nsl],
                                             start=(hl == 0),
                                             stop=(hl == HPC - 1))
                        oo = epool.tile([128, 512], F16, tag="oo", bufs=4)
                        if nb % 2 == 0:
                            nc.vector.tensor_copy(oo[:], ps[:])
                            nc.sync.dma_start(
                                out[ch * CS + m01 * 128:
                                    ch * CS + m01 * 128 + 128, nsl], oo[:])
                        else:
                            nc.scalar.copy(oo[:], ps[:])
                            nc.scalar.dma_start(
                                out[ch * CS + m01 * 128:
                                    ch * CS + m01 * 128 + 128, nsl], oo[:])

    nc.compile()
    return nc


def _prep_inputs(x, c, Wq, Wk, Wv, Wg, Wgt, Wo):
    """Build the 8 per-core input maps (host-side sharding / layout)."""
    consts = np.ascontiguousarray(_consts_np())
    c16 = np.concatenate(
        [np.eye(128, dtype=np.float16), np.ones((128, 1), np.float16)], axis=1)
    in_maps = []
    xTs = [np.ascontiguousarray(x[b].T.astype(np.float16)) for b in range(B)]
    xc = x + c
    xcTs = [np.ascontiguousarray(xc[b].T.astype(np.float16)) for b in range(B)]
    for core in range(NCORE):
        b, g = core // 4, core % 4
        cols = slice(g * PCOLS, (g + 1) * PCOLS)
        in_maps.append({
            "xT": xTs[b],
            "xcT": np.ascontiguousarray(xcTs[b][cols, :]),
            "wq": np.ascontiguousarray(Wq[:, cols]).astype(np.float16),
            "wk": np.ascontiguousarray(Wk[:, cols]).astype(np.float16),
            "wv": np.ascontiguousarray(Wv[:, cols]).astype(np.float16),
            "wg": np.ascontiguousarray(Wg[:, cols]).astype(np.float16),
            "wgt": np.ascontiguousarray(Wgt[cols, :]).astype(np.float16),
            "wo": np.ascontiguousarray(Wo[cols, :]).astype(np.float16),
            "consts": consts,
            "c16": c16,
        })
    return in_maps


def kernel(x, c, Wq, Wk, Wv, Wg, Wgt, Wo, _want_results=False):
    key = "nc_dbg" if DEBUG else "nc"
    if key not in _cache:
        _cache[key] = build(debug=DEBUG)
    nc = _cache[key]
    in_maps = _prep_inputs(np.asarray(x, np.float32), np.asarray(c, np.float32),
                           np.asarray(Wq, np.float32), np.asarray(Wk, np.float32),
                           np.asarray(Wv, np.float32), np.asarray(Wg, np.float32),
                           np.asarray(Wgt, np.float32), np.asarray(Wo, np.float32))
    res = bass_utils.run_bass_kernel_spmd(
        nc, in_maps, core_ids=list(range(NCORE)), trace=TRACE)
    out = np.zeros((B, T, DIM), np.float32)
    for core in range(NCORE):
        out[core // 4] += res.results[core]["out"].astype(np.float32)
    if _want_results:
        return out, res
    return out


# revision 9
# speedup vs baseline: 1.1607x; 1.1607x over previous
"""GateRetention Trainium2 kernel (Bass/Tile), 8-core tensor-parallel.

Sharding: core grid (batch b = core//4, head-group g = core%4); each core owns
4 heads (512 cols of the q/k/v/g projections, 512 rows of Wo) of one batch.
RMS-norm statistics are AllReduced across each batch's 4 cores; gate logits
are contraction-sharded (host pre-adds x+c, each core contracts a 512-row
quarter for all 16 heads) and ReduceScattered so each core gets its 4 heads.
Out-proj partials are summed on the host (row-parallel TP gather).

Precision: all projections and retention in fp16 with fp32 PSUM accumulation;
a 2^-2 exponent shift on vfac keeps decayed v tiles in fp16 range.  The
rowfac (per-token decay * q-norm * scale) is folded into the subln norm
scale f = rf * rsqrt(rf^2 * sumsq/HD + eps), so the raw retention output is
normalized+scaled in one pass (exactly equal to norm(rf*o_raw)).

kernel(**inputs) takes the FULL inputs from reference.setup_inputs() and
returns the FULL [B, T, DIM] fp32 output.
"""
import os
import sys

sys.path.insert(0, "/opt/trn_rl_repo")

import numpy as np

import concourse.bass as bass
import concourse.bacc as bacc
import concourse.tile as tile
import concourse.mybir as mybir
from concourse import bass_utils

F32 = mybir.dt.float32
F32R = mybir.dt.float32r
F16 = mybir.dt.float16
AX = mybir.AxisListType
ALU = mybir.AluOpType
ACTF = mybir.ActivationFunctionType

B, T, DIM = 2, 4096, 2048
H, HD = 16, 128
CS = 256
NCH = T // CS              # 16 chunks
EPS = 1e-5
GLN = 16.0
SCALE = HD ** -0.5
NCORE = 8
HPC = 4                    # heads per core
PCOLS = HPC * HD           # 512 cols per core
NBLK = T // 128            # 32 token blocks of 128
NT = T // 512              # 8 token n-tiles
VSH = 2.0 ** -2            # fp16 range shift on vv; inverse folded into rowfac

DEBUG_LVL = int(os.environ.get("GR_DEBUG", "0"))
DEBUG = bool(DEBUG_LVL)
TRACE = bool(int(os.environ.get("GR_TRACE", "0")))
PET = not bool(int(os.environ.get("GR_XPT", "0")))  # PE transpose for o_n
KNPE = bool(int(os.environ.get("GR_KNPE", "0")))    # PE-transpose fallback for kn

_cache = {}


def _consts_np():
    """[128, 904] fp32: identity | Lm | Om | Um | ones8 | Lc | M0."""
    ident = np.eye(128, dtype=np.float32)
    jj, ii = np.meshgrid(np.arange(128), np.arange(128), indexing="ij")
    Lm = np.where(jj <= ii, -1.0 / GLN, 0.0).astype(np.float32)
    Om = np.full((128, 128), -1.0 / GLN, np.float32)
    Um = np.where(jj <= ii, 1.0, 0.0).astype(np.float32)
    ones = np.ones((128, 8), np.float32)
    # Lc: b_i - b_mid for block0 = +1/GLN * sum_{j>i} sp_j
    Lc = np.where(jj > ii, 1.0 / GLN, 0.0).astype(np.float32)
    M0 = np.concatenate([Um, np.ones((128, 128), np.float32)], axis=1)
    return np.concatenate([ident, Lm, Om, Um, ones, Lc, M0], axis=1)


def build(debug=False):
    nc = bacc.Bacc("TRN2", target_bir_lowering=False, debug=False,
                   enable_asserts=False, num_devices=NCORE)

    # ---------------- I/O ----------------
    xT = nc.dram_tensor("xT", [DIM, T], F16, kind="ExternalInput").ap()
    # (x+c)^T row-quarter for this core's contraction shard of the gate logits
    xcT = nc.dram_tensor("xcT", [PCOLS, T], F16, kind="ExternalInput").ap()
    wq = nc.dram_tensor("wq", [DIM, PCOLS], F16, kind="ExternalInput").ap()
    wk = nc.dram_tensor("wk", [DIM, PCOLS], F16, kind="ExternalInput").ap()
    wv = nc.dram_tensor("wv", [DIM, PCOLS], F16, kind="ExternalInput").ap()
    wg = nc.dram_tensor("wg", [DIM, PCOLS], F16, kind="ExternalInput").ap()
    wgt = nc.dram_tensor("wgt", [PCOLS, H], F16, kind="ExternalInput").ap()
    wo = nc.dram_tensor("wo", [PCOLS, DIM], F16, kind="ExternalInput").ap()
    consts = nc.dram_tensor("consts", [128, 904], F32R, kind="ExternalInput").ap()
    c16 = nc.dram_tensor("c16", [128, 129], F16, kind="ExternalInput").ap()
    out = nc.dram_tensor("out", [T, DIM], F16, kind="ExternalOutput").ap()

    def dbg(name, shape, dtype=F32):
        return nc.dram_tensor(name, shape, dtype, kind="ExternalOutput").ap()

    with tile.TileContext(nc) as tc:
        with (
            tc.tile_pool(name="const", bufs=1) as cpool,
            tc.tile_pool(name="wts", bufs=1) as wpool,
            tc.tile_pool(name="xstream", bufs=4) as xpool,
            tc.tile_pool(name="cstream", bufs=2) as ctpool,
            tc.tile_pool(name="evac", bufs=4) as epool,
            tc.tile_pool(name="persist", bufs=1) as ppool,
            tc.tile_pool(name="small", bufs=2) as spool,
            tc.tile_pool(name="ret", bufs=2) as rpool,
            tc.tile_pool(name="ps", bufs=1, space="PSUM") as psp,
            tc.tile_pool(name="dram", bufs=1, space="DRAM") as dpool,
        ):
            def ps_big():
                return psp.tile([128, 512], F32, tag="big", bufs=2, name="psbig")

            def ps_small(shape=None, dtype=F32):
                return psp.tile(shape or [128, 256], dtype, tag="small", bufs=2,
                                name="pssmall")

            def ps_oraw():
                return psp.tile([128, 2 * HPC, HD], F32, tag="oraw", bufs=2,
                                name="psoraw")

            # ---------------- constants ----------------
            cst = cpool.tile([128, 904], F32R, tag="consts")
            nc.sync.dma_start(cst[:], consts)
            ident32 = cst[:, 0:128].bitcast(F32)
            Lm = cst[:, 128:256]
            Om = cst[:, 256:384]
            Um32 = cst[:, 384:512].bitcast(F32)
            Lc = cst[:, 520:648]
            M0 = cst[:, 648:904].bitcast(F32)
            aux16 = cpool.tile([128, 129], F16, tag="i16")
            nc.sync.dma_start(aux16[:], c16)
            i16 = aux16[:, 0:128]
            ones16 = aux16[:, 128:129]

            # ---------------- DRAM scratch (fp16) ----------------
            if debug:
                qT_s = dbg("dbg_qT", [PCOLS, T], F16)
                kT_s = dbg("dbg_kT", [PCOLS, T], F16)
                vN_s = dbg("dbg_vN", [T, PCOLS], F16)
                gT_s = dbg("dbg_gT", [PCOLS, T], F16)
            else:
                qT_s = nc.dram_tensor("qT_s", [PCOLS, T], F16,
                                      kind="Internal").ap()
                kT_s = nc.dram_tensor("kT_s", [PCOLS, T], F16,
                                      kind="Internal").ap()
                vN_s = nc.dram_tensor("vN_s", [T, PCOLS], F16,
                                      kind="Internal").ap()
                gT_s = nc.dram_tensor("gT_s", [PCOLS, T], F16,
                                      kind="Internal").ap()
            ss_in = nc.dram_tensor("ss_in", [3, T], F32, kind="Internal").ap()
            ss_out = nc.dram_tensor("ss_out", [3, T], F32, kind="Internal").ap()
            gt_in = nc.dram_tensor("gt_in", [H, T], F32, kind="Internal").ap()
            gt_out = nc.dram_tensor("gt_out", [HPC, T], F32,
                                    kind="Internal").ap()

            # =========================================================
            # P1: single fused pass: q,k,g (T-layout), v (natural),
            # gate-logit partials; all fp16 math, fp32 PSUM.
            # =========================================================
            vss = ppool.tile([128, NBLK], F32, tag="vss")

            def load_w(wdram, tag):
                wt = wpool.tile([128, 16, 512], F16, tag=tag)
                nc.sync.dma_start(
                    wt[:], wdram.rearrange("(kt p) m -> p kt m", p=128))
                return wt

            wq_sb = load_w(wq, "w0")
            wk_sb = load_w(wk, "w1")
            wv_sb = load_w(wv, "w2")
            wg_sb = load_w(wg, "w3")
            wgt_sb = wpool.tile([128, HPC, H], F16, tag="wgt")
            nc.sync.dma_start(wgt_sb[:],
                              wgt.rearrange("(kt p) m -> p kt m", p=128))

            def tproj_mms(ps, wt, xth, m):
                for k in range(16):
                    nc.tensor.matmul(
                        ps[:], wt[:, k, m * 128:(m + 1) * 128],
                        xth[k // 8][:, k % 8, :], start=(k == 0), stop=(k == 15))

            for n in range(NT):
                tok = slice(n * 512, (n + 1) * 512)
                xth = []
                for h2 in range(2):
                    xt = xpool.tile([128, 8, 512], F16, tag="xt")
                    nc.sync.dma_start(
                        xt[:], xT[h2 * 1024:(h2 + 1) * 1024, tok].rearrange(
                            "(kt p) m -> p kt m", p=128))
                    xth.append(xt)
                # ---- q, k: T-layout + per-token sumsq over this core's cols
                for pi, (wsb, sdram) in enumerate(((wq_sb, qT_s), (wk_sb, kT_s))):
                    for m in range(4):
                        ps = ps_big()
                        tproj_mms(ps, wsb, xth, m)
                        ev = epool.tile([128, 512], F16, tag="ev")
                        sqt = epool.tile([128, 512], F16, tag="sq")
                        if m % 2 == 0:
                            nc.vector.tensor_copy(ev[:], ps[:])
                            nc.scalar.activation(sqt[:], ps[:], ACTF.Square)
                            nc.sync.dma_start(
                                sdram[m * 128:(m + 1) * 128, tok], ev[:])
                        else:
                            nc.scalar.copy(ev[:], ps[:])
                            nc.vector.tensor_tensor(sqt[:], ev[:], ev[:],
                                                    ALU.mult)
                            nc.scalar.dma_start(
                                sdram[m * 128:(m + 1) * 128, tok], ev[:])
                        if m == 0:
                            ssps = ps_small([1, 512])
                        nc.tensor.matmul(ssps[:1, :], ones16, sqt[:],
                                         start=(m == 0), stop=(m == 3))
                        if m == 3:
                            ssev = spool.tile([1, 512], F32, tag="ssev", bufs=2)
                            nc.vector.tensor_copy(ssev[:], ssps[:1, :])
                            nc.sync.dma_start(ss_in[pi:pi + 1, tok], ssev[:])
                # ---- g: silu, T-layout
                for m in range(4):
                    ps = ps_big()
                    tproj_mms(ps, wg_sb, xth, m)
                    ev = epool.tile([128, 512], F16, tag="ev")
                    nc.scalar.activation(ev[:], ps[:], ACTF.Silu)
                    nc.scalar.dma_start(gT_s[m * 128:(m + 1) * 128, tok], ev[:])
                # ---- v: natural layout + accumulated sumsq
                for mt in range(4):
                    msl = slice(mt * 128, (mt + 1) * 128)
                    ps = ps_big()
                    for k in range(16):
                        nc.tensor.matmul(
                            ps[:], xth[k // 8][:, k % 8, msl], wv_sb[:, k, :],
                            start=(k == 0), stop=(k == 15))
                    ev = epool.tile([128, 512], F16, tag="ev")
                    sqt = epool.tile([128, 512], F16, tag="sq")
                    nc.vector.tensor_copy(ev[:], ps[:])
                    nc.scalar.activation(
                        sqt[:], ps[:], ACTF.Square,
                        accum_out=vss[:, n * 4 + mt:n * 4 + mt + 1])
                    nc.sync.dma_start(
                        vN_s[n * 512 + mt * 128:n * 512 + (mt + 1) * 128, :],
                        ev[:])
                # ---- gate logits: contraction-sharded over (x+c) quarter
                ct = ctpool.tile([128, HPC, 512], F16, tag="ct")
                nc.sync.dma_start(
                    ct[:], xcT[:, tok].rearrange("(kt p) m -> p kt m", p=128))
                gtps = ps_small([H, 512])
                for kk in range(HPC):
                    nc.tensor.matmul(gtps[:H, :], wgt_sb[:, kk, :],
                                     ct[:, kk, :], start=(kk == 0),
                                     stop=(kk == HPC - 1))
                gstg = spool.tile([H, 512], F32, tag="gstg", bufs=2)
                nc.vector.tensor_copy(gstg[:], gtps[:H, :])
                nc.scalar.dma_start(gt_in[:, tok], gstg[:])

            # v sumsq: transpose [128, 32] -> [32, 128] -> ss_in row 2
            vssT = ps_small([128, 128])
            nc.tensor.matmul(vssT[:32, :], vss[:], ident32, is_transpose=True)
            vssev = spool.tile([32, 128], F32, tag="vssev", bufs=1)
            nc.vector.tensor_copy(vssev[:], vssT[:32, :])
            nc.sync.dma_start(
                ss_in[2:3, :].rearrange("a (b c) -> (a b) c", c=128), vssev[:])

            # =========================================================
            # P2: collectives; scales; gate decays
            # =========================================================
            nc.gpsimd.collective_compute(
                "AllReduce", ALU.add,
                replica_groups=[[0, 1, 2, 3], [4, 5, 6, 7]],
                ins=[ss_in.opt()], outs=[ss_out.opt()],
            )
            nc.gpsimd.collective_compute(
                "ReduceScatter", ALU.add,
                replica_groups=[[0, 1, 2, 3], [4, 5, 6, 7]],
                ins=[gt_in.opt()], outs=[gt_out.opt()],
            )
            # transpose [7, T] (3 sumsq + 4 gate-logit rows) into token-major
            ssgt = ppool.tile([128, NBLK, 7], F32, tag="ssgt")
            for nn_ in range(NT):
                tok = slice(nn_ * 512, (nn_ + 1) * 512)
                srt = spool.tile([7, 512], F32, tag="srt", bufs=2)
                nc.sync.dma_start(srt[0:3, :], ss_out[:, tok])
                nc.sync.dma_start(srt[3:7, :], gt_out[:, tok])
                for j in range(4):
                    tp = ps_small([128, 8])
                    nc.tensor.matmul(tp[:, :7], srt[:, j * 128:(j + 1) * 128],
                                     ident32[:7, :7], is_transpose=True)
                    nc.vector.tensor_copy(ssgt[:, nn_ * 4 + j, :], tp[:, :7])
            rsn = ppool.tile([128, NBLK, 3], F32, tag="rsn")
            nc.vector.tensor_scalar(rsn[:], ssgt[:, :, 0:3], 1.0 / DIM, EPS,
                                    ALU.mult, ALU.add)
            nc.scalar.activation(rsn[:], rsn[:], ACTF.Ln)
            nc.scalar.activation(rsn[:], rsn[:], ACTF.Exp, scale=-0.5)
            skv = ppool.tile([128, NBLK], F32, tag="skv")
            nc.vector.tensor_mul(skv[:], rsn[:, :, 1], rsn[:, :, 2])
            if debug and DEBUG_LVL >= 2:
                nc.sync.dma_start(dbg("dbg_rsn", [128, NBLK * 3]),
                                  rsn[:].rearrange("p a b -> p (a b)"))

            # gate decays: sp = softplus(-z) = ln(1 + exp(-z)); -1/GLN in Lm/Om
            gtd = ppool.tile([128, NBLK, HPC], F32R, tag="gtd")
            gtn = ppool.tile([128, NBLK, HPC], F32, tag="gtn")
            nc.scalar.activation(gtn[:], ssgt[:, :, 3:7], ACTF.Exp, scale=-1.0)
            nc.scalar.activation(gtd[:], gtn[:], ACTF.Ln, bias=1.0)

            # per chunk: recentered b' = b - b_mid via triangular matmuls;
            # eS = exp(mid-to-mid decay) for the state recurrence
            rf = ppool.tile([128, NCH, 2, HPC], F32, tag="rf")      # rowfac
            vf = ppool.tile([128, NCH, 2, HPC], F32, tag="vf")      # vfac
            eS = ppool.tile([128, NCH, HPC], F32, tag="eS")
            for ch in range(NCH):
                b0, b1 = 2 * ch, 2 * ch + 1
                p0 = ps_small([128, HPC])
                nc.tensor.matmul(p0[:], Lc, gtd[:, b0, :], start=True, stop=True)
                p1 = ps_small([128, HPC])
                nc.tensor.matmul(p1[:], Lm, gtd[:, b1, :], start=True, stop=True)
                if ch < NCH - 1:
                    pt = ps_small([128, HPC])
                    nc.tensor.matmul(pt[:], Om, gtd[:, b1, :],
                                     start=True, stop=False)
                    nc.tensor.matmul(pt[:], Om, gtd[:, b1 + 1, :],
                                     start=False, stop=True)
                    nc.scalar.activation(eS[:, ch, :], pt[:], ACTF.Exp)
                for blk01, bps in ((0, p0), (1, p1)):
                    blk = 2 * ch + blk01
                    # rowfac = exp(b') * sq * scale / VSH
                    nc.scalar.activation(rf[:, ch, blk01, :], bps[:], ACTF.Exp)
                    nc.vector.tensor_scalar(
                        rf[:, ch, blk01, :], rf[:, ch, blk01, :],
                        rsn[:, blk, 0:1], SCALE / VSH, ALU.mult, ALU.mult)
                    # vfac = exp(-b') * sk * sv * VSH
                    nc.scalar.activation(vf[:, ch, blk01, :], bps[:], ACTF.Exp,
                                         scale=-1.0)
                    nc.vector.tensor_scalar(
                        vf[:, ch, blk01, :], vf[:, ch, blk01, :],
                        skv[:, blk:blk + 1], VSH, ALU.mult, ALU.mult)
            # rf2 = rf^2 / HD (for the fused norm scale)
            rf2 = ppool.tile([128, NCH, 2, HPC], F32, tag="rf2")
            nc.vector.scalar_tensor_tensor(
                rf2[:], rf[:], 1.0 / HD, rf[:], op0=ALU.mult, op1=ALU.mult)

            if debug and DEBUG_LVL >= 3:
                nc.sync.dma_start(
                    dbg("dbg_rf", [128, NCH * 2 * HPC]),
                    rf[:].rearrange("p a b c -> p (a b c)"))
                nc.sync.dma_start(
                    dbg("dbg_vf", [128, NCH * 2 * HPC]),
                    vf[:].rearrange("p a b c -> p (a b c)"))
                nc.sync.dma_start(
                    dbg("dbg_eS", [128, NCH * HPC]),
                    eS[:].rearrange("p a b -> p (a b)"))
            if debug and DEBUG_LVL >= 2:
                nc.sync.dma_start(
                    dbg("dbg_gtd", [128, NBLK * HPC]),
                    gtd[:].bitcast(F32).rearrange("p a b -> p (a b)"))

            # =========================================================
            # P3: retention + fused norm/gate + out-proj, per chunk
            # =========================================================
            if int(os.environ.get("GR_BARRIER", "0")):
                tc.prologue_barrier()
            wo_sb = wpool.tile([128, HPC, DIM], F16, tag="wo")
            nc.sync.dma_start(wo_sb[:], wo.rearrange("(h p) m -> p h m", p=128))

            S_prev = None
            for ch in range(NCH):
                tok = slice(ch * CS, (ch + 1) * CS)
                qc = rpool.tile([128, HPC, CS], F16, tag="qc")
                kc = rpool.tile([128, HPC, CS], F16, tag="kc")
                for t_, s_ in ((qc, qT_s), (kc, kT_s)):
                    nc.sync.dma_start(
                        t_[:], s_[:, tok].rearrange("(h p) m -> p h m", p=128))
                # k natural layout: one batched XBAR transpose per token block
                kn = []
                if ch < NCH - 1:
                    if KNPE:
                        for hl in range(HPC):
                            for ci in range(2):
                                tpk = ps_small([128, 128], F16)
                                nc.tensor.transpose(
                                    tpk[:], kc[:, hl,
                                               ci * 128:ci * 128 + 128], i16)
                                knt = rpool.tile([128, 128], F16,
                                                 tag=f"kn{hl * 2 + ci}")
                                if (hl + ci) % 2 == 0:
                                    nc.scalar.copy(knt[:], tpk[:])
                                else:
                                    nc.vector.tensor_copy(knt[:], tpk[:])
                                kn.append(knt)
                    else:
                        for ci in range(2):
                            bt = slice(ch * CS + ci * 128,
                                       ch * CS + ci * 128 + 128)
                            knb = rpool.tile([128, PCOLS], F16,
                                             tag=f"knb{ci}")
                            nc.scalar.dma_start_transpose(
                                knb[:], kT_s[:, bt])
                            kn.append(knb)
                vcn = []
                for ci in range(2):
                    bt = slice(ch * CS + ci * 128, ch * CS + ci * 128 + 128)
                    vt = rpool.tile([128, PCOLS], F16, tag="vcn", bufs=4)
                    nc.scalar.dma_start(vt[:], vN_s[bt, :])
                    vcn.append(vt)
                sg = rpool.tile([128, HPC, CS], F16, tag="sg")
                nc.sync.dma_start(
                    sg[:], gT_s[:, tok].rearrange("(h p) m -> p h m", p=128))
                # vv = v * vfac
                vvt = rpool.tile([128, 2, HPC, HD], F16, tag="vvt")
                for ci in range(2):
                    for hl in range(HPC):
                        nc.vector.tensor_scalar(
                            vvt[:, ci, hl, :],
                            vcn[ci][:, hl * 128:(hl + 1) * 128],
                            vf[:, ch, ci, hl:hl + 1], None, ALU.mult)
                # AT (masked): rows tj, cols ti
                at0s, at1s = [], []
                for hl in range(HPC):
                    at0ps = ps_small([128, 256])
                    nc.tensor.matmul(at0ps[:], kc[:, hl, 0:128], qc[:, hl, :],
                                     start=True, stop=True)
                    at0 = rpool.tile([128, CS], F16, tag="at0", bufs=4)
                    nc.vector.scalar_tensor_tensor(
                        at0[:], at0ps[:], 1.0, M0, op0=ALU.mult, op1=ALU.mult)
                    at0s.append(at0)
                    at1ps = ps_small([128, 128])
                    nc.tensor.matmul(at1ps[:], kc[:, hl, 128:256],
                                     qc[:, hl, 128:256], start=True, stop=True)
                    at1 = rpool.tile([128, 128], F16, tag="at1s", bufs=4)
                    nc.vector.scalar_tensor_tensor(
                        at1[:], at1ps[:], 1.0, Um32, op0=ALU.mult, op1=ALU.mult)
                    at1s.append(at1)
                # o_raw = intra + inter, regions (ci*HPC + hl)
                orps = ps_oraw()
                for hl in range(HPC):
                    for ci in range(2):
                        reg = orps[:, ci * HPC + hl, :]
                        mms = [(at0s[hl][:, ci * 128:ci * 128 + 128],
                                vvt[:, 0, hl, :])]
                        if ci == 1:
                            mms.append((at1s[hl][:], vvt[:, 1, hl, :]))
                        if ch > 0:
                            mms.append((qc[:, hl, ci * 128:ci * 128 + 128],
                                        S_prev[:, hl, :]))
                        for i, (lh, rh) in enumerate(mms):
                            nc.tensor.matmul(reg, lh, rh, start=(i == 0),
                                             stop=(i == len(mms) - 1))
                # state update: S_cur = (S_prev + kn^T vv) * eS
                if ch < NCH - 1:
                    sps = ps_small([128, HPC, HD])
                    for hl in range(HPC):
                        hsl = slice(hl * 128, (hl + 1) * 128)
                        kn0 = kn[hl * 2][:] if KNPE else kn[0][:, hsl]
                        kn1 = kn[hl * 2 + 1][:] if KNPE else kn[1][:, hsl]
                        nc.tensor.matmul(sps[:, hl, :], kn0,
                                         vvt[:, 0, hl, :], start=True,
                                         stop=False)
                        nc.tensor.matmul(sps[:, hl, :], kn1,
                                         vvt[:, 1, hl, :], start=False,
                                         stop=True)
                    eSb = eS[:, ch, :].unsqueeze(2).to_broadcast(
                        [128, HPC, HD])
                    S_cur = rpool.tile([128, HPC, HD], F16, tag="S")
                    if ch > 0:
                        stmp = rpool.tile([128, HPC, HD], F32, tag="stmp")
                        nc.vector.tensor_tensor(stmp[:], sps[:], S_prev[:],
                                                ALU.add)
                        nc.vector.tensor_tensor(S_cur[:], stmp[:], eSb,
                                                ALU.mult)
                    else:
                        nc.vector.tensor_tensor(S_cur[:], sps[:], eSb,
                                                ALU.mult)
                    S_prev = S_cur
                # fused subln norm + rowfac: f = rf*rsqrt(rf^2*ss/HD + eps)
                ssum = rpool.tile([128, 2 * HPC], F32, tag="ssum")
                for idx in range(2 * HPC):
                    osq = rpool.tile([128, HD], F32, tag="osq", bufs=4)
                    nc.scalar.activation(osq[:], orps[:, idx, :], ACTF.Square,
                                         accum_out=ssum[:, idx:idx + 1])
                rfv = rf[:, ch].rearrange("p a b -> p (a b)")
                rf2v = rf2[:, ch].rearrange("p a b -> p (a b)")
                dd = rpool.tile([128, 2 * HPC], F32, tag="dd")
                nc.vector.tensor_tensor(dd[:], rf2v, ssum[:], ALU.mult)
                nc.vector.tensor_scalar(dd[:], dd[:], EPS, None, ALU.add)
                nc.scalar.activation(dd[:], dd[:], ACTF.Ln)
                nc.scalar.activation(dd[:], dd[:], ACTF.Exp, scale=-0.5)
                ff = rpool.tile([128, 2 * HPC], F32, tag="ff")
                nc.vector.tensor_tensor(ff[:], rfv, dd[:], ALU.mult)
                o_n = rpool.tile([128, 2 * HPC, HD], F16, tag="o_n")
                for half in range(2):
                    hsl = slice(half * HPC, (half + 1) * HPC)
                    nc.vector.tensor_tensor(
                        o_n[:, hsl, :], orps[:, hsl, :],
                        ff[:, hsl].unsqueeze(2).to_broadcast([128, HPC, HD]),
                        ALU.mult)
                # transpose to [chan, tok] + gate (idx = ci*HPC + hl)
                go = []
                for ci in range(2):
                    for hl in range(HPC):
                        idx = ci * HPC + hl
                        got = rpool.tile([128, 128], F16, tag=f"go{idx}")
                        if PET:
                            trp = ps_small([128, 128], F16)
                            nc.tensor.transpose(trp[:], o_n[:, idx, :], i16)
                            nc.vector.tensor_mul(
                                got[:], trp[:],
                                sg[:, hl, ci * 128:ci * 128 + 128])
                        else:
                            tro = rpool.tile([128, 128], F16, tag=f"tr{idx}")
                            nc.sync.dma_start_transpose(
                                tro[:], o_n[:, idx, :])
                            nc.vector.tensor_mul(
                                got[:], tro[:],
                                sg[:, hl, ci * 128:ci * 128 + 128])
                        go.append(got)
                # out-proj for this chunk's two token tiles
                for m01 in range(2):
                    for nb in range(DIM // 512):
                        ps = ps_big()
                        nsl = slice(nb * 512, (nb + 1) * 512)
                        for hl in range(HPC):
                            nc.tensor.matmul(ps[:], go[m01 * HPC + hl][:],
                                             wo_sb[:, hl, nsl],
                                             start=(hl == 0),
                                             stop=(hl == HPC - 1))
                        oo = epool.tile([128, 512], F16, tag="oo", bufs=4)
                        if nb % 2 == 0:
                            nc.vector.tensor_copy(oo[:], ps[:])
                            nc.sync.dma_start(
                                out[ch * CS + m01 * 128:
                                    ch * CS + m01 * 128 + 128, nsl], oo[:])
                        else:
                            nc.scalar.copy(oo[:], ps[:])
                            nc.scalar.dma_start(
                                out[ch * CS + m01 * 128:
                                    ch * CS + m01 * 128 + 128, nsl], oo[:])

    nc.compile()
    return nc


def _prep_inputs(x, c, Wq, Wk, Wv, Wg, Wgt, Wo):
    """Build the 8 per-core input maps (host-side sharding / layout)."""
    consts = np.ascontiguousarray(_consts_np())
    c16 = np.concatenate(
        [np.eye(128, dtype=np.float16), np.ones((128, 1), np.float16)], axis=1)
    in_maps = []
    xTs = [np.ascontiguousarray(x[b].T.astype(np.float16)) for b in range(B)]
    xc = x + c
    xcTs = [np.ascontiguousarray(xc[b].T.astype(np.float16)) for b in range(B)]
    for core in range(NCORE):
        b, g = core // 4, core % 4
        cols = slice(g * PCOLS, (g + 1) * PCOLS)
        in_maps.append({
            "xT": xTs[b],
            "xcT": np.ascontiguousarray(xcTs[b][cols, :]),
            "wq": np.ascontiguousarray(Wq[:, cols]).astype(np.float16),
            "wk": np.ascontiguousarray(Wk[:, cols]).astype(np.float16),
            "wv": np.ascontiguousarray(Wv[:, cols]).astype(np.float16),
            "wg": np.ascontiguousarray(Wg[:, cols]).astype(np.float16),
            "wgt": np.ascontiguousarray(Wgt[cols, :]).astype(np.float16),
            "wo": np.ascontiguousarray(Wo[cols, :]).astype(np.float16),
            "consts": consts,
            "c16": c16,
        })
    return in_maps


def kernel(x, c, Wq, Wk, Wv, Wg, Wgt, Wo, _want_results=False):
    key = "nc_dbg" if DEBUG else "nc"
    if key not in _cache:
        _cache[key] = build(debug=DEBUG)
    nc = _cache[key]
    in_maps = _prep_inputs(np.asarray(x, np.float32), np.asarray(c, np.float32),
                           np.asarray(Wq, np.float32), np.asarray(Wk, np.float32),
                           np.asarray(Wv, np.float32), np.asarray(Wg, np.float32),
                           np.asarray(Wgt, np.float32), np.asarray(Wo, np.float32))
    res = bass_utils.run_bass_kernel_spmd(
        nc, in_maps, core_ids=list(range(NCORE)), trace=TRACE)
    out = np.zeros((B, T, DIM), np.float32)
    for core in range(NCORE):
        out[core // 4] += res.results[core]["out"].astype(np.float32)
    if _want_results:
        return out, res
    return out


# revision 22
# speedup vs baseline: 1.3770x; 1.1864x over previous
"""GateRetention Trainium2 kernel (Bass/Tile), 8-core tensor-parallel.

Sharding: core grid (batch b = core//4, head-group g = core%4); each core owns
4 heads (512 cols of the q/k/v/g projections, 512 rows of Wo) of one batch.
RMS-norm statistics are AllReduced across each batch's 4 cores; gate logits
are contraction-sharded (host pre-adds x+c, each core contracts a 512-row
quarter for all 16 heads) and ReduceScattered so each core gets its 4 heads.
Collectives are split per half-T and issued mid-P1 so they overlap compute.
Out-proj partials are summed on the host (row-parallel TP gather).

Precision: all projections and retention in fp16 with fp32 PSUM accumulation;
a 2^-2 exponent shift on vfac keeps decayed v tiles in fp16 range.  The
rowfac (per-token decay * q-norm * scale) is folded into the subln norm
scale f = rf * rsqrt(rf^2 * sumsq/HD + eps), so the raw retention output is
normalized+scaled in one pass (exactly equal to norm(rf*o_raw)).

kernel(**inputs) takes the FULL inputs from reference.setup_inputs() and
returns the FULL [B, T, DIM] fp32 output.
"""
import os
import sys

sys.path.insert(0, "/opt/trn_rl_repo")

import numpy as np

import concourse.bass as bass
import concourse.bacc as bacc
import concourse.tile as tile
import concourse.mybir as mybir
from concourse import bass_utils

F32 = mybir.dt.float32
F32R = mybir.dt.float32r
F16 = mybir.dt.float16
AX = mybir.AxisListType
ALU = mybir.AluOpType
ACTF = mybir.ActivationFunctionType

B, T, DIM = 2, 4096, 2048
H, HD = 16, 128
CS = 256
NCH = T // CS              # 16 chunks
EPS = 1e-5
GLN = 16.0
SCALE = HD ** -0.5
NCORE = 8
HPC = 4                    # heads per core
PCOLS = HPC * HD           # 512 cols per core
NBLK = T // 128            # 32 token blocks of 128
NT = T // 512              # 8 token n-tiles
TH = T // 2                # tokens per collective half
VSH = 2.0 ** -2            # fp16 range shift on vv; inverse folded into rowfac

DEBUG_LVL = int(os.environ.get("GR_DEBUG", "0"))
DEBUG = bool(DEBUG_LVL)
TRACE = bool(int(os.environ.get("GR_TRACE", "0")))
PET = not bool(int(os.environ.get("GR_XPT", "0")))  # PE transpose for o_n
KNPE = bool(int(os.environ.get("GR_KNPE", "0")))    # PE-transpose fallback for kn

_cache = {}


def _consts_np():
    """[128, 904] fp32: identity | Lm | Om | Um | ones8 | Lc | M0."""
    ident = np.eye(128, dtype=np.float32)
    jj, ii = np.meshgrid(np.arange(128), np.arange(128), indexing="ij")
    Lm = np.where(jj <= ii, -1.0 / GLN, 0.0).astype(np.float32)
    Om = np.full((128, 128), -1.0 / GLN, np.float32)
    Um = np.where(jj <= ii, 1.0, 0.0).astype(np.float32)
    ones = np.ones((128, 8), np.float32)
    ones[:, 1] = EPS                       # col 513: eps bias for Ln
    # Lc: b_i - b_mid for block0 = +1/GLN * sum_{j>i} sp_j
    Lc = np.where(jj > ii, 1.0 / GLN, 0.0).astype(np.float32)
    M0 = np.concatenate([Um, np.ones((128, 128), np.float32)], axis=1)
    return np.concatenate([ident, Lm, Om, Um, ones, Lc, M0], axis=1)


def build(debug=False):
    nc = bacc.Bacc("TRN2", target_bir_lowering=False, debug=False,
                   enable_asserts=False, num_devices=NCORE)

    # ---------------- I/O ----------------
    xT = nc.dram_tensor("xT", [DIM, T], F16, kind="ExternalInput").ap()
    # (x+c)^T row-quarter for this core's contraction shard of the gate logits
    xcT = nc.dram_tensor("xcT", [PCOLS, T], F16, kind="ExternalInput").ap()
    wq = nc.dram_tensor("wq", [DIM, PCOLS], F16, kind="ExternalInput").ap()
    wk = nc.dram_tensor("wk", [DIM, PCOLS], F16, kind="ExternalInput").ap()
    wv = nc.dram_tensor("wv", [DIM, PCOLS], F16, kind="ExternalInput").ap()
    wg = nc.dram_tensor("wg", [DIM, PCOLS], F16, kind="ExternalInput").ap()
    wgt = nc.dram_tensor("wgt", [PCOLS, H], F16, kind="ExternalInput").ap()
    wo = nc.dram_tensor("wo", [PCOLS, DIM], F16, kind="ExternalInput").ap()
    consts = nc.dram_tensor("consts", [128, 904], F32R, kind="ExternalInput").ap()
    c16 = nc.dram_tensor("c16", [128, 129], F16, kind="ExternalInput").ap()
    out = nc.dram_tensor("out", [T, DIM], F16, kind="ExternalOutput").ap()

    def dbg(name, shape, dtype=F32):
        return nc.dram_tensor(name, shape, dtype, kind="ExternalOutput").ap()

    with tile.TileContext(nc) as tc:
        with (
            tc.tile_pool(name="const", bufs=1) as cpool,
            tc.tile_pool(name="wts", bufs=1) as wpool,
            tc.tile_pool(name="xstream", bufs=4) as xpool,
            tc.tile_pool(name="cstream", bufs=2) as ctpool,
            tc.tile_pool(name="evac", bufs=4) as epool,
            tc.tile_pool(name="persist", bufs=1) as ppool,
            tc.tile_pool(name="small", bufs=2) as spool,
            tc.tile_pool(name="ret", bufs=2) as rpool,
            tc.tile_pool(name="ps", bufs=1, space="PSUM") as psp,
            tc.tile_pool(name="dram", bufs=1, space="DRAM") as dpool,
        ):
            def ps_big():
                return psp.tile([128, 512], F32, tag="big", bufs=2, name="psbig")

            def ps_small(shape=None, dtype=F32):
                return psp.tile(shape or [128, 256], dtype, tag="small", bufs=2,
                                name="pssmall")

            def ps_oraw():
                return psp.tile([128, HPC, HD], F32, tag="oraw", bufs=2,
                                name="psoraw")

            def ps_trp():
                return psp.tile([128, 128], F16, tag="trp", bufs=2,
                                name="pstrp")

            # ---------------- constants ----------------
            cst = cpool.tile([128, 904], F32R, tag="consts")
            nc.sync.dma_start(cst[:], consts)
            ident32 = cst[:, 0:128].bitcast(F32)
            Lm = cst[:, 128:256]
            Om = cst[:, 256:384]
            Um32 = cst[:, 384:512].bitcast(F32)
            epsb = cst[:, 513:514].bitcast(F32)
            Lc = cst[:, 520:648]
            M0 = cst[:, 648:904].bitcast(F32)
            aux16 = cpool.tile([128, 129], F16, tag="i16")
            nc.sync.dma_start(aux16[:], c16)
            i16 = aux16[:, 0:128]
            ones16 = aux16[:, 128:129]

            # ---------------- DRAM scratch (fp16) ----------------
            if debug:
                qT_s = dbg("dbg_qT", [PCOLS, T], F16)
                kT_s = dbg("dbg_kT", [PCOLS, T], F16)
                vN_s = dbg("dbg_vN", [T, PCOLS], F16)
                gT_s = dbg("dbg_gT", [PCOLS, T], F16)
            else:
                qT_s = nc.dram_tensor("qT_s", [PCOLS, T], F16,
                                      kind="Internal").ap()
                kT_s = nc.dram_tensor("kT_s", [PCOLS, T], F16,
                                      kind="Internal").ap()
                vN_s = nc.dram_tensor("vN_s", [T, PCOLS], F16,
                                      kind="Internal").ap()
                gT_s = nc.dram_tensor("gT_s", [PCOLS, T], F16,
                                      kind="Internal").ap()
            ss_in = [nc.dram_tensor(f"ss_in{h}", [3, TH], F32,
                                    kind="Internal").ap() for h in range(2)]
            ss_out = [nc.dram_tensor(f"ss_out{h}", [3, TH], F32,
                                     kind="Internal").ap() for h in range(2)]
            gt_in = [nc.dram_tensor(f"gt_in{h}", [H, TH], F32,
                                    kind="Internal").ap() for h in range(2)]
            gt_out = [nc.dram_tensor(f"gt_out{h}", [HPC, TH], F32,
                                     kind="Internal").ap() for h in range(2)]

            # =========================================================
            # P1: single fused pass: q,k,g (T-layout), v (natural),
            # gate-logit partials; all fp16 math, fp32 PSUM.
            # Collectives per half-T, issued mid-stream so they overlap.
            # =========================================================
            vss = ppool.tile([128, NBLK], F32, tag="vss")

            def load_w(wdram, tag):
                wt = wpool.tile([128, 16, 512], F16, tag=tag)
                nc.sync.dma_start(
                    wt[:], wdram.rearrange("(kt p) m -> p kt m", p=128))
                return wt

            wq_sb = load_w(wq, "w0")
            wk_sb = load_w(wk, "w1")
            wv_sb = load_w(wv, "w2")
            wg_sb = load_w(wg, "w3")
            wgt_sb = wpool.tile([128, HPC, H], F16, tag="wgt")
            nc.sync.dma_start(wgt_sb[:],
                              wgt.rearrange("(kt p) m -> p kt m", p=128))

            def tproj_mms(ps, wt, xth, m):
                for k in range(16):
                    nc.tensor.matmul(
                        ps[:], wt[:, k, m * 128:(m + 1) * 128],
                        xth[k // 8][:, k % 8, :], start=(k == 0), stop=(k == 15))

            for half in range(2):
                for n in range(half * 4, half * 4 + 4):
                    tok = slice(n * 512, (n + 1) * 512)
                    ltok = slice(n * 512 - half * TH, (n + 1) * 512 - half * TH)
                    xth = []
                    for h2 in range(2):
                        xt = xpool.tile([128, 8, 512], F16, tag="xt")
                        nc.sync.dma_start(
                            xt[:], xT[h2 * 1024:(h2 + 1) * 1024, tok].rearrange(
                                "(kt p) m -> p kt m", p=128))
                        xth.append(xt)
                    # -- q, k: T-layout; squares kept for deferred sumsq
                    sqs = {0: [], 1: []}
                    for pi, (wsb, sdram) in enumerate(((wq_sb, qT_s),
                                                       (wk_sb, kT_s))):
                        for m in range(4):
                            ps = ps_big()
                            tproj_mms(ps, wsb, xth, m)
                            ev = epool.tile([128, 512], F16, tag="ev")
                            sqt = epool.tile([128, 512], F16, tag="sq",
                                             bufs=10)
                            if m % 2 == 0:
                                nc.vector.tensor_copy(ev[:], ps[:])
                                nc.scalar.activation(sqt[:], ps[:],
                                                     ACTF.Square)
                                nc.sync.dma_start(
                                    sdram[m * 128:(m + 1) * 128, tok], ev[:])
                            else:
                                nc.scalar.copy(ev[:], ps[:])
                                nc.vector.tensor_tensor(sqt[:], ev[:], ev[:],
                                                        ALU.mult)
                                nc.scalar.dma_start(
                                    sdram[m * 128:(m + 1) * 128, tok], ev[:])
                            sqs[pi].append(sqt)
                    # -- g: silu, T-layout
                    for m in range(4):
                        ps = ps_big()
                        tproj_mms(ps, wg_sb, xth, m)
                        ev = epool.tile([128, 512], F16, tag="ev")
                        nc.scalar.activation(ev[:], ps[:], ACTF.Silu)
                        nc.scalar.dma_start(gT_s[m * 128:(m + 1) * 128, tok],
                                            ev[:])
                    # -- v: natural layout + accumulated sumsq
                    for mt in range(4):
                        msl = slice(mt * 128, (mt + 1) * 128)
                        ps = ps_big()
                        for k in range(16):
                            nc.tensor.matmul(
                                ps[:], xth[k // 8][:, k % 8, msl],
                                wv_sb[:, k, :], start=(k == 0), stop=(k == 15))
                        ev = epool.tile([128, 512], F16, tag="ev")
                        sqt = epool.tile([128, 512], F16, tag="vsq", bufs=2)
                        nc.vector.tensor_copy(ev[:], ps[:])
                        nc.scalar.activation(
                            sqt[:], ps[:], ACTF.Square,
                            accum_out=vss[:, n * 4 + mt:n * 4 + mt + 1])
                        nc.sync.dma_start(
                            vN_s[n * 512 + mt * 128:n * 512 + (mt + 1) * 128,
                                 :], ev[:])
                    # -- gate logits: contraction-sharded over (x+c) quarter
                    ct = ctpool.tile([128, HPC, 512], F16, tag="ct")
                    nc.sync.dma_start(
                        ct[:], xcT[:, tok].rearrange("(kt p) m -> p kt m",
                                                     p=128))
                    gtps = ps_small([H, 512])
                    for kk in range(HPC):
                        nc.tensor.matmul(gtps[:H, :], wgt_sb[:, kk, :],
                                         ct[:, kk, :], start=(kk == 0),
                                         stop=(kk == HPC - 1))
                    gstg = spool.tile([H, 512], F32, tag="gstg", bufs=2)
                    nc.vector.tensor_copy(gstg[:], gtps[:H, :])
                    nc.scalar.dma_start(gt_in[half][:, ltok], gstg[:])
                    # -- deferred sumsq matmuls (inputs long since evacuated)
                    for pi in range(2):
                        ssps = ps_small([1, 512])
                        for m in range(4):
                            nc.tensor.matmul(ssps[:1, :], ones16,
                                             sqs[pi][m][:], start=(m == 0),
                                             stop=(m == 3))
                        ssev = spool.tile([1, 512], F32, tag="ssev", bufs=2)
                        nc.vector.tensor_copy(ssev[:], ssps[:1, :])
                        nc.sync.dma_start(ss_in[half][pi:pi + 1, ltok],
                                          ssev[:])
                # v sumsq for this half: transpose [128,16] -> row 2
                vssT = ps_small([128, 128])
                nc.tensor.matmul(vssT[:16, :],
                                 vss[:, half * 16:(half + 1) * 16], ident32,
                                 is_transpose=True)
                vssev = spool.tile([16, 128], F32, tag="vssev", bufs=2)
                nc.vector.tensor_copy(vssev[:], vssT[:16, :])
                nc.sync.dma_start(
                    ss_in[half][2:3, :].rearrange("a (b c) -> (a b) c", c=128),
                    vssev[:])
                # collectives for this half (overlap with the next half's P1)
                nc.gpsimd.collective_compute(
                    "AllReduce", ALU.add,
                    replica_groups=[[0, 1, 2, 3], [4, 5, 6, 7]],
                    ins=[ss_in[half].opt()], outs=[ss_out[half].opt()],
                )
                nc.gpsimd.collective_compute(
                    "ReduceScatter", ALU.add,
                    replica_groups=[[0, 1, 2, 3], [4, 5, 6, 7]],
                    ins=[gt_in[half].opt()], outs=[gt_out[half].opt()],
                )

            # =========================================================
            # P2 (per half): norm scales + gate decays
            # =========================================================
            ssgt = ppool.tile([128, NBLK, 7], F32, tag="ssgt")
            rsn = ppool.tile([128, NBLK, 3], F32, tag="rsn")
            skv = ppool.tile([128, NBLK], F32, tag="skv")
            gtd = ppool.tile([128, NBLK, HPC], F32R, tag="gtd")
            gtn = ppool.tile([128, NBLK, HPC], F32, tag="gtn")
            rf = ppool.tile([128, NCH, 2, HPC], F32, tag="rf")      # rowfac
            vf = ppool.tile([128, NCH, 2, HPC], F32, tag="vf")      # vfac
            eS = ppool.tile([128, NCH, HPC], F32, tag="eS")
            rf2 = ppool.tile([128, NCH, 2, HPC], F32, tag="rf2")

            def es_part(ch):
                # eS[ch] couples chunk ch and ch+1 (blocks 2ch+1, 2ch+2)
                b1 = 2 * ch + 1
                pt = ps_small([128, HPC])
                nc.tensor.matmul(pt[:], Om, gtd[:, b1, :], start=True,
                                 stop=False)
                nc.tensor.matmul(pt[:], Om, gtd[:, b1 + 1, :], start=False,
                                 stop=True)
                nc.scalar.activation(eS[:, ch, :], pt[:], ACTF.Exp)

            def p2_half(half):
                hb = slice(half * 16, (half + 1) * 16)
                for nn_ in range(4):
                    ltok = slice(nn_ * 512, (nn_ + 1) * 512)
                    srt = spool.tile([7, 512], F32, tag="srt", bufs=2)
                    nc.sync.dma_start(srt[0:3, :], ss_out[half][:, ltok])
                    nc.sync.dma_start(srt[3:7, :], gt_out[half][:, ltok])
                    for j in range(4):
                        tp = ps_small([128, 8])
                        nc.tensor.matmul(tp[:, :7],
                                         srt[:, j * 128:(j + 1) * 128],
                                         ident32[:7, :7], is_transpose=True)
                        nc.vector.tensor_copy(
                            ssgt[:, half * 16 + nn_ * 4 + j, :], tp[:, :7])
                nc.vector.tensor_scalar(rsn[:, hb], ssgt[:, hb, 0:3],
                                        1.0 / DIM, EPS, ALU.mult, ALU.add)
                nc.scalar.activation(rsn[:, hb], rsn[:, hb], ACTF.Ln)
                nc.scalar.activation(rsn[:, hb], rsn[:, hb], ACTF.Exp,
                                     scale=-0.5)
                nc.vector.tensor_mul(skv[:, hb], rsn[:, hb, 1], rsn[:, hb, 2])
                nc.scalar.activation(gtn[:, hb], ssgt[:, hb, 3:7], ACTF.Exp,
                                     scale=-1.0)
                nc.scalar.activation(gtd[:, hb], gtn[:, hb], ACTF.Ln, bias=1.0)
                if half == 1:
                    es_part(7)  # needs block 16 (half 1), deferred to here
                for ch in range(half * 8, half * 8 + 8):
                    b0, b1 = 2 * ch, 2 * ch + 1
                    p0 = ps_small([128, HPC])
                    nc.tensor.matmul(p0[:], Lc, gtd[:, b0, :], start=True,
                                     stop=True)
                    p1 = ps_small([128, HPC])
                    nc.tensor.matmul(p1[:], Lm, gtd[:, b1, :], start=True,
                                     stop=True)
                    if ch < NCH - 1 and ch != 7:
                        es_part(ch)
                    for blk01, bps in ((0, p0), (1, p1)):
                        blk = 2 * ch + blk01
                        # rowfac = exp(b') * sq * scale / VSH
                        nc.scalar.activation(rf[:, ch, blk01, :], bps[:],
                                             ACTF.Exp)
                        nc.vector.tensor_scalar(
                            rf[:, ch, blk01, :], rf[:, ch, blk01, :],
                            rsn[:, blk, 0:1], SCALE / VSH, ALU.mult, ALU.mult)
                        # vfac = exp(-b') * sk * sv * VSH
                        nc.scalar.activation(vf[:, ch, blk01, :], bps[:],
                                             ACTF.Exp, scale=-1.0)
                        nc.vector.tensor_scalar(
                            vf[:, ch, blk01, :], vf[:, ch, blk01, :],
                            skv[:, blk:blk + 1], VSH, ALU.mult, ALU.mult)
                # rf2 = rf^2 / HD (for the fused norm scale)
                nc.vector.scalar_tensor_tensor(
                    rf2[:, half * 8:(half + 1) * 8],
                    rf[:, half * 8:(half + 1) * 8],
                    1.0 / HD, rf[:, half * 8:(half + 1) * 8],
                    op0=ALU.mult, op1=ALU.mult)

            # =========================================================
            # P3: retention + fused norm/gate + out-proj, per chunk
            # =========================================================
            if int(os.environ.get("GR_BARRIER", "0")):
                tc.prologue_barrier()
            wo_sb = wpool.tile([128, HPC, DIM], F16, tag="wo")
            nc.sync.dma_start(wo_sb[:], wo.rearrange("(h p) m -> p h m", p=128))

            S_box = [None]

            def p3_chunk(ch):
                S_prev = S_box[0]
                tok = slice(ch * CS, (ch + 1) * CS)
                qc = rpool.tile([128, HPC, CS], F16, tag="qc")
                kc = rpool.tile([128, HPC, CS], F16, tag="kc")
                for t_, s_ in ((qc, qT_s), (kc, kT_s)):
                    nc.sync.dma_start(
                        t_[:], s_[:, tok].rearrange("(h p) m -> p h m", p=128))
                # k natural layout: one batched XBAR transpose per token block
                kn = []
                if ch < NCH - 1:
                    if KNPE:
                        for hl in range(HPC):
                            for ci in range(2):
                                tpk = ps_trp()
                                nc.tensor.transpose(
                                    tpk[:], kc[:, hl,
                                               ci * 128:ci * 128 + 128], i16)
                                knt = rpool.tile([128, 128], F16,
                                                 tag=f"kn{hl * 2 + ci}")
                                if (hl + ci) % 2 == 0:
                                    nc.scalar.copy(knt[:], tpk[:])
                                else:
                                    nc.vector.tensor_copy(knt[:], tpk[:])
                                kn.append(knt)
                    else:
                        for ci in range(2):
                            bt = slice(ch * CS + ci * 128,
                                       ch * CS + ci * 128 + 128)
                            knb = rpool.tile([128, PCOLS], F16,
                                             tag=f"knb{ci}")
                            nc.scalar.dma_start_transpose(
                                knb[:], kT_s[:, bt])
                            kn.append(knb)
                vcn = []
                for ci in range(2):
                    bt = slice(ch * CS + ci * 128, ch * CS + ci * 128 + 128)
                    vt = rpool.tile([128, PCOLS], F16, tag="vcn", bufs=4)
                    nc.scalar.dma_start(vt[:], vN_s[bt, :])
                    vcn.append(vt)
                sg = rpool.tile([128, HPC, CS], F16, tag="sg")
                nc.sync.dma_start(
                    sg[:], gT_s[:, tok].rearrange("(h p) m -> p h m", p=128))
                # vv = v * vfac
                vvt = rpool.tile([128, 2, HPC, HD], F16, tag="vvt")
                for ci in range(2):
                    for hl in range(HPC):
                        nc.vector.tensor_scalar(
                            vvt[:, ci, hl, :],
                            vcn[ci][:, hl * 128:(hl + 1) * 128],
                            vf[:, ch, ci, hl:hl + 1], None, ALU.mult)
                # AT (masked): rows tj, cols ti
                at0s, at1s = [], []
                for hl in range(HPC):
                    at0ps = ps_small([128, 256])
                    nc.tensor.matmul(at0ps[:], kc[:, hl, 0:128], qc[:, hl, :],
                                     start=True, stop=True)
                    at0 = rpool.tile([128, CS], F16, tag="at0", bufs=4)
                    nc.vector.scalar_tensor_tensor(
                        at0[:], at0ps[:], 1.0, M0, op0=ALU.mult, op1=ALU.mult)
                    at0s.append(at0)
                    at1ps = ps_small([128, 128])
                    nc.tensor.matmul(at1ps[:], kc[:, hl, 128:256],
                                     qc[:, hl, 128:256], start=True, stop=True)
                    at1 = rpool.tile([128, 128], F16, tag="at1s", bufs=4)
                    nc.vector.scalar_tensor_tensor(
                        at1[:], at1ps[:], 1.0, Um32, op0=ALU.mult,
                        op1=ALU.mult)
                    at1s.append(at1)
                # o_raw = intra + inter; one PSUM tile per token half-block
                orp = []
                for ci in range(2):
                    orps = ps_oraw()
                    for hl in range(HPC):
                        reg = orps[:, hl, :]
                        mms = [(at0s[hl][:, ci * 128:ci * 128 + 128],
                                vvt[:, 0, hl, :])]
                        if ci == 1:
                            mms.append((at1s[hl][:], vvt[:, 1, hl, :]))
                        if ch > 0:
                            mms.append((qc[:, hl, ci * 128:ci * 128 + 128],
                                        S_prev[:, hl, :]))
                        for i, (lh, rh) in enumerate(mms):
                            nc.tensor.matmul(reg, lh, rh, start=(i == 0),
                                             stop=(i == len(mms) - 1))
                    orp.append(orps)
                # state update: S_cur = (S_prev + kn^T vv) * eS
                if ch < NCH - 1:
                    sps = ps_small([128, HPC, HD])
                    for hl in range(HPC):
                        hsl = slice(hl * 128, (hl + 1) * 128)
                        kn0 = kn[hl * 2][:] if KNPE else kn[0][:, hsl]
                        kn1 = kn[hl * 2 + 1][:] if KNPE else kn[1][:, hsl]
                        nc.tensor.matmul(sps[:, hl, :], kn0,
                                         vvt[:, 0, hl, :], start=True,
                                         stop=False)
                        nc.tensor.matmul(sps[:, hl, :], kn1,
                                         vvt[:, 1, hl, :], start=False,
                                         stop=True)
                    eSb = eS[:, ch, :].unsqueeze(2).to_broadcast(
                        [128, HPC, HD])
                    S_cur = rpool.tile([128, HPC, HD], F16, tag="S")
                    if ch > 0:
                        stmp = rpool.tile([128, HPC, HD], F32, tag="stmp")
                        nc.vector.tensor_tensor(stmp[:], sps[:], S_prev[:],
                                                ALU.add)
                        nc.vector.tensor_tensor(S_cur[:], stmp[:], eSb,
                                                ALU.mult)
                    else:
                        nc.vector.tensor_tensor(S_cur[:], sps[:], eSb,
                                                ALU.mult)
                    S_box[0] = S_cur
                # fused subln norm + rowfac: f = rf*rsqrt(rf^2*ss/HD + eps)
                ssum = rpool.tile([128, 2 * HPC], F32, tag="ssum")
                for idx in range(2 * HPC):
                    ci, hl = idx // HPC, idx % HPC
                    osq = rpool.tile([128, HD], F32, tag="osq", bufs=4)
                    nc.scalar.activation(osq[:], orp[ci][:, hl, :],
                                         ACTF.Square,
                                         accum_out=ssum[:, idx:idx + 1])
                rfv = rf[:, ch].rearrange("p a b -> p (a b)")
                rf2v = rf2[:, ch].rearrange("p a b -> p (a b)")
                dd = rpool.tile([128, 2 * HPC], F32, tag="dd")
                nc.vector.tensor_tensor(dd[:], rf2v, ssum[:], ALU.mult)
                nc.scalar.activation(dd[:], dd[:], ACTF.Ln, bias=epsb)
                nc.scalar.activation(dd[:], dd[:], ACTF.Exp, scale=-0.5)
                ff = rpool.tile([128, 2 * HPC], F32, tag="ff")
                nc.vector.tensor_tensor(ff[:], rfv, dd[:], ALU.mult)
                o_n = rpool.tile([128, 2 * HPC, HD], F16, tag="o_n")
                for ci in range(2):
                    hsl = slice(ci * HPC, (ci + 1) * HPC)
                    nc.vector.tensor_tensor(
                        o_n[:, hsl, :], orp[ci][:],
                        ff[:, hsl].unsqueeze(2).to_broadcast([128, HPC, HD]),
                        ALU.mult)
                # transpose to [chan, tok] + gate (idx = ci*HPC + hl)
                go = []
                for ci in range(2):
                    for hl in range(HPC):
                        idx = ci * HPC + hl
                        got = rpool.tile([128, 128], F16, tag=f"go{idx}")
                        if PET:
                            trp = ps_trp()
                            nc.tensor.transpose(trp[:], o_n[:, idx, :], i16)
                            nc.vector.tensor_mul(
                                got[:], trp[:],
                                sg[:, hl, ci * 128:ci * 128 + 128])
                        else:
                            tro = rpool.tile([128, 128], F16, tag=f"tr{idx}")
                            nc.sync.dma_start_transpose(
                                tro[:], o_n[:, idx, :])
                            nc.vector.tensor_mul(
                                got[:], tro[:],
                                sg[:, hl, ci * 128:ci * 128 + 128])
                        go.append(got)
                # out-proj for this chunk's two token tiles
                for m01 in range(2):
                    for nb in range(DIM // 512):
                        ps = ps_big()
                        nsl = slice(nb * 512, (nb + 1) * 512)
                        for hl in range(HPC):
                            nc.tensor.matmul(ps[:], go[m01 * HPC + hl][:],
                                             wo_sb[:, hl, nsl],
                                             start=(hl == 0),
                                             stop=(hl == HPC - 1))
                        oo = epool.tile([128, 512], F16, tag="oo", bufs=4)
                        if nb % 2 == 0:
                            nc.vector.tensor_copy(oo[:], ps[:])
                            nc.sync.dma_start(
                                out[ch * CS + m01 * 128:
                                    ch * CS + m01 * 128 + 128, nsl], oo[:])
                        else:
                            nc.scalar.copy(oo[:], ps[:])
                            nc.scalar.dma_start(
                                out[ch * CS + m01 * 128:
                                    ch * CS + m01 * 128 + 128, nsl], oo[:])

            # emission order: P2 half0 right after P1 (its collective
            # completed mid-P1), first two chunks, then P2 half1 (its
            # collective completes around P1 end), then the rest.
            p2_half(0)
            p3_chunk(0)
            p3_chunk(1)
            p2_half(1)
            for ch in range(2, NCH):
                p3_chunk(ch)

            if debug and DEBUG_LVL >= 3:
                nc.sync.dma_start(
                    dbg("dbg_rf", [128, NCH * 2 * HPC]),
                    rf[:].rearrange("p a b c -> p (a b c)"))
                nc.sync.dma_start(
                    dbg("dbg_vf", [128, NCH * 2 * HPC]),
                    vf[:].rearrange("p a b c -> p (a b c)"))
                nc.sync.dma_start(
                    dbg("dbg_eS", [128, NCH * HPC]),
                    eS[:].rearrange("p a b -> p (a b)"))
            if debug and DEBUG_LVL >= 2:
                nc.sync.dma_start(
                    dbg("dbg_rsn", [128, NBLK * 3]),
                    rsn[:].rearrange("p a b -> p (a b)"))
                nc.sync.dma_start(
                    dbg("dbg_gtd", [128, NBLK * HPC]),
                    gtd[:].bitcast(F32).rearrange("p a b -> p (a b)"))

    nc.compile()
    return nc


def _prep_inputs(x, c, Wq, Wk, Wv, Wg, Wgt, Wo):
    """Build the 8 per-core input maps (host-side sharding / layout)."""
    consts = np.ascontiguousarray(_consts_np())
    c16 = np.concatenate(
        [np.eye(128, dtype=np.float16), np.ones((128, 1), np.float16)], axis=1)
    in_maps = []
    xTs = [np.ascontiguousarray(x[b].T.astype(np.float16)) for b in range(B)]
    xc = x + c
    xcTs = [np.ascontiguousarray(xc[b].T.astype(np.float16)) for b in range(B)]
    for core in range(NCORE):
        b, g = core // 4, core % 4
        cols = slice(g * PCOLS, (g + 1) * PCOLS)
        in_maps.append({
            "xT": xTs[b],
            "xcT": np.ascontiguousarray(xcTs[b][cols, :]),
            "wq": np.ascontiguousarray(Wq[:, cols]).astype(np.float16),
            "wk": np.ascontiguousarray(Wk[:, cols]).astype(np.float16),
            "wv": np.ascontiguousarray(Wv[:, cols]).astype(np.float16),
            "wg": np.ascontiguousarray(Wg[:, cols]).astype(np.float16),
            "wgt": np.ascontiguousarray(Wgt[cols, :]).astype(np.float16),
            "wo": np.ascontiguousarray(Wo[cols, :]).astype(np.float16),
            "consts": consts,
            "c16": c16,
        })
    return in_maps


def kernel(x, c, Wq, Wk, Wv, Wg, Wgt, Wo, _want_results=False):
    key = "nc_dbg" if DEBUG else "nc"
    if key not in _cache:
        _cache[key] = build(debug=DEBUG)
    nc = _cache[key]
    in_maps = _prep_inputs(np.asarray(x, np.float32), np.asarray(c, np.float32),
                           np.asarray(Wq, np.float32), np.asarray(Wk, np.float32),
                           np.asarray(Wv, np.float32), np.asarray(Wg, np.float32),
                           np.asarray(Wgt, np.float32), np.asarray(Wo, np.float32))
    res = bass_utils.run_bass_kernel_spmd(
        nc, in_maps, core_ids=list(range(NCORE)), trace=TRACE)
    out = np.zeros((B, T, DIM), np.float32)
    for core in range(NCORE):
        out[core // 4] += res.results[core]["out"].astype(np.float32)
    if _want_results:
        return out, res
    return out


# revision 24
# speedup vs baseline: 1.3998x; 1.0165x over previous
"""GateRetention Trainium2 kernel (Bass/Tile), 8-core tensor-parallel.

Sharding: core grid (batch b = core//4, head-group g = core%4); each core owns
4 heads (512 cols of the q/k/v/g projections, 512 rows of Wo) of one batch.
RMS-norm statistics are AllReduced across each batch's 4 cores; gate logits
are contraction-sharded (host pre-adds x+c, each core contracts a 512-row
quarter for all 16 heads) and ReduceScattered so each core gets its 4 heads.
Collectives are split per half-T and issued mid-P1 so they overlap compute.
Out-proj partials are summed on the host (row-parallel TP gather).

Precision: all projections and retention in fp16 with fp32 PSUM accumulation;
a 2^-2 exponent shift on vfac keeps decayed v tiles in fp16 range.  The
rowfac (per-token decay * q-norm * scale) is folded into the subln norm
scale f = rf * rsqrt(rf^2 * sumsq/HD + eps), so the raw retention output is
normalized+scaled in one pass (exactly equal to norm(rf*o_raw)).

kernel(**inputs) takes the FULL inputs from reference.setup_inputs() and
returns the FULL [B, T, DIM] fp32 output.
"""
import os
import sys

sys.path.insert(0, "/opt/trn_rl_repo")

import numpy as np

import concourse.bass as bass
import concourse.bacc as bacc
import concourse.tile as tile
import concourse.mybir as mybir
from concourse import bass_utils

F32 = mybir.dt.float32
F32R = mybir.dt.float32r
F16 = mybir.dt.float16
AX = mybir.AxisListType
ALU = mybir.AluOpType
ACTF = mybir.ActivationFunctionType

B, T, DIM = 2, 4096, 2048
H, HD = 16, 128
CS = 256
NCH = T // CS              # 16 chunks
EPS = 1e-5
GLN = 16.0
SCALE = HD ** -0.5
NCORE = 8
HPC = 4                    # heads per core
PCOLS = HPC * HD           # 512 cols per core
NBLK = T // 128            # 32 token blocks of 128
NT = T // 512              # 8 token n-tiles
TH = T // 2                # tokens per collective half
VSH = 2.0 ** -2            # fp16 range shift on vv; inverse folded into rowfac

DEBUG_LVL = int(os.environ.get("GR_DEBUG", "0"))
DEBUG = bool(DEBUG_LVL)
TRACE = bool(int(os.environ.get("GR_TRACE", "0")))
PET = not bool(int(os.environ.get("GR_XPT", "0")))  # PE transpose for o_n
KNPE = bool(int(os.environ.get("GR_KNPE", "0")))    # PE-transpose fallback for kn

_cache = {}


def _consts_np():
    """[128, 904] fp32: identity | Lm | Om | Um | ones8 | Lc | M0."""
    ident = np.eye(128, dtype=np.float32)
    jj, ii = np.meshgrid(np.arange(128), np.arange(128), indexing="ij")
    Lm = np.where(jj <= ii, -1.0 / GLN, 0.0).astype(np.float32)
    Om = np.full((128, 128), -1.0 / GLN, np.float32)
    Um = np.where(jj <= ii, 1.0, 0.0).astype(np.float32)
    ones = np.ones((128, 8), np.float32)
    ones[:, 1] = EPS                       # col 513: eps bias for Ln
    # Lc: b_i - b_mid for block0 = +1/GLN * sum_{j>i} sp_j
    Lc = np.where(jj > ii, 1.0 / GLN, 0.0).astype(np.float32)
    M0 = np.concatenate([Um, np.ones((128, 128), np.float32)], axis=1)
    return np.concatenate([ident, Lm, Om, Um, ones, Lc, M0], axis=1)


def build(debug=False):
    nc = bacc.Bacc("TRN2", target_bir_lowering=False, debug=False,
                   enable_asserts=False, num_devices=NCORE)

    # ---------------- I/O ----------------
    xT = nc.dram_tensor("xT", [DIM, T], F16, kind="ExternalInput").ap()
    # (x+c)^T row-quarter for this core's contraction shard of the gate logits
    xcT = nc.dram_tensor("xcT", [PCOLS, T], F16, kind="ExternalInput").ap()
    wq = nc.dram_tensor("wq", [DIM, PCOLS], F16, kind="ExternalInput").ap()
    wk = nc.dram_tensor("wk", [DIM, PCOLS], F16, kind="ExternalInput").ap()
    wv = nc.dram_tensor("wv", [DIM, PCOLS], F16, kind="ExternalInput").ap()
    wg = nc.dram_tensor("wg", [DIM, PCOLS], F16, kind="ExternalInput").ap()
    wgt = nc.dram_tensor("wgt", [PCOLS, H], F16, kind="ExternalInput").ap()
    wo = nc.dram_tensor("wo", [PCOLS, DIM], F16, kind="ExternalInput").ap()
    consts = nc.dram_tensor("consts", [128, 904], F32R, kind="ExternalInput").ap()
    c16 = nc.dram_tensor("c16", [128, 129], F16, kind="ExternalInput").ap()
    out = nc.dram_tensor("out", [T, DIM], F16, kind="ExternalOutput").ap()

    def dbg(name, shape, dtype=F32):
        return nc.dram_tensor(name, shape, dtype, kind="ExternalOutput").ap()

    with tile.TileContext(nc) as tc:
        with (
            tc.tile_pool(name="const", bufs=1) as cpool,
            tc.tile_pool(name="wts", bufs=1) as wpool,
            tc.tile_pool(name="xstream", bufs=4) as xpool,
            tc.tile_pool(name="cstream", bufs=2) as ctpool,
            tc.tile_pool(name="evac", bufs=4) as epool,
            tc.tile_pool(name="persist", bufs=1) as ppool,
            tc.tile_pool(name="small", bufs=2) as spool,
            tc.tile_pool(name="ret", bufs=2) as rpool,
            tc.tile_pool(name="ps", bufs=1, space="PSUM") as psp,
            tc.tile_pool(name="dram", bufs=1, space="DRAM") as dpool,
        ):
            def ps_big():
                return psp.tile([128, 512], F32, tag="big", bufs=2, name="psbig")

            def ps_small(shape=None, dtype=F32):
                return psp.tile(shape or [128, 256], dtype, tag="small", bufs=2,
                                name="pssmall")

            def ps_oraw():
                return psp.tile([128, HPC, HD], F32, tag="oraw", bufs=2,
                                name="psoraw")

            def ps_trp():
                return psp.tile([128, 128], F16, tag="trp", bufs=2,
                                name="pstrp")

            # ---------------- constants ----------------
            cst = cpool.tile([128, 904], F32R, tag="consts")
            nc.sync.dma_start(cst[:], consts)
            ident32 = cst[:, 0:128].bitcast(F32)
            Lm = cst[:, 128:256]
            Om = cst[:, 256:384]
            Um32 = cst[:, 384:512].bitcast(F32)
            epsb = cst[:, 513:514].bitcast(F32)
            Lc = cst[:, 520:648]
            M0 = cst[:, 648:904].bitcast(F32)
            aux16 = cpool.tile([128, 129], F16, tag="i16")
            nc.sync.dma_start(aux16[:], c16)
            i16 = aux16[:, 0:128]
            ones16 = aux16[:, 128:129]

            # ---------------- DRAM scratch (fp16) ----------------
            if debug:
                qT_s = dbg("dbg_qT", [PCOLS, T], F16)
                kT_s = dbg("dbg_kT", [PCOLS, T], F16)
                vN_s = dbg("dbg_vN", [T, PCOLS], F16)
                gT_s = dbg("dbg_gT", [PCOLS, T], F16)
            else:
                qT_s = nc.dram_tensor("qT_s", [PCOLS, T], F16,
                                      kind="Internal").ap()
                kT_s = nc.dram_tensor("kT_s", [PCOLS, T], F16,
                                      kind="Internal").ap()
                vN_s = nc.dram_tensor("vN_s", [T, PCOLS], F16,
                                      kind="Internal").ap()
                gT_s = nc.dram_tensor("gT_s", [PCOLS, T], F16,
                                      kind="Internal").ap()
            ss_in = [nc.dram_tensor(f"ss_in{h}", [3, TH], F32,
                                    kind="Internal").ap() for h in range(2)]
            ss_out = [nc.dram_tensor(f"ss_out{h}", [3, TH], F32,
                                     kind="Internal").ap() for h in range(2)]
            gt_in = [nc.dram_tensor(f"gt_in{h}", [H, TH], F32,
                                    kind="Internal").ap() for h in range(2)]
            gt_out = [nc.dram_tensor(f"gt_out{h}", [HPC, TH], F32,
                                     kind="Internal").ap() for h in range(2)]

            # =========================================================
            # P1: single fused pass: q,k,g (T-layout), v (natural),
            # gate-logit partials; all fp16 math, fp32 PSUM.
            # Collectives per half-T, issued mid-stream so they overlap.
            # =========================================================
            vss = ppool.tile([128, NBLK], F32, tag="vss")

            def load_w(wdram, tag):
                wt = wpool.tile([128, 16, 512], F16, tag=tag)
                nc.sync.dma_start(
                    wt[:], wdram.rearrange("(kt p) m -> p kt m", p=128))
                return wt

            wq_sb = load_w(wq, "w0")
            wk_sb = load_w(wk, "w1")
            wv_sb = load_w(wv, "w2")
            wg_sb = load_w(wg, "w3")
            wgt_sb = wpool.tile([128, HPC, H], F16, tag="wgt")
            nc.sync.dma_start(wgt_sb[:],
                              wgt.rearrange("(kt p) m -> p kt m", p=128))

            def tproj_mms(ps, wt, xth, m):
                for k in range(16):
                    nc.tensor.matmul(
                        ps[:], wt[:, k, m * 128:(m + 1) * 128],
                        xth[k // 8][:, k % 8, :], start=(k == 0), stop=(k == 15))

            for half in range(2):
                for n in range(half * 4, half * 4 + 4):
                    tok = slice(n * 512, (n + 1) * 512)
                    ltok = slice(n * 512 - half * TH, (n + 1) * 512 - half * TH)
                    xth = []
                    for h2 in range(2):
                        xt = xpool.tile([128, 8, 512], F16, tag="xt")
                        nc.sync.dma_start(
                            xt[:], xT[h2 * 1024:(h2 + 1) * 1024, tok].rearrange(
                                "(kt p) m -> p kt m", p=128))
                        xth.append(xt)
                    # -- q, k: T-layout; squares kept for deferred sumsq
                    sqs = {0: [], 1: []}
                    for pi, (wsb, sdram) in enumerate(((wq_sb, qT_s),
                                                       (wk_sb, kT_s))):
                        for m in range(4):
                            ps = ps_big()
                            tproj_mms(ps, wsb, xth, m)
                            ev = epool.tile([128, 512], F16, tag="ev")
                            sqt = epool.tile([128, 512], F16, tag="sq",
                                             bufs=10)
                            if m % 2 == 0:
                                nc.vector.tensor_copy(ev[:], ps[:])
                                nc.scalar.activation(sqt[:], ps[:],
                                                     ACTF.Square)
                                nc.sync.dma_start(
                                    sdram[m * 128:(m + 1) * 128, tok], ev[:])
                            else:
                                nc.scalar.copy(ev[:], ps[:])
                                nc.vector.tensor_tensor(sqt[:], ev[:], ev[:],
                                                        ALU.mult)
                                nc.scalar.dma_start(
                                    sdram[m * 128:(m + 1) * 128, tok], ev[:])
                            sqs[pi].append(sqt)
                    # -- g: silu, T-layout
                    for m in range(4):
                        ps = ps_big()
                        tproj_mms(ps, wg_sb, xth, m)
                        ev = epool.tile([128, 512], F16, tag="ev")
                        nc.scalar.activation(ev[:], ps[:], ACTF.Silu)
                        nc.scalar.dma_start(gT_s[m * 128:(m + 1) * 128, tok],
                                            ev[:])
                    # -- v: natural layout + accumulated sumsq
                    for mt in range(4):
                        msl = slice(mt * 128, (mt + 1) * 128)
                        ps = ps_big()
                        for k in range(16):
                            nc.tensor.matmul(
                                ps[:], xth[k // 8][:, k % 8, msl],
                                wv_sb[:, k, :], start=(k == 0), stop=(k == 15))
                        ev = epool.tile([128, 512], F16, tag="ev")
                        sqt = epool.tile([128, 512], F16, tag="vsq", bufs=2)
                        nc.vector.tensor_copy(ev[:], ps[:])
                        nc.scalar.activation(
                            sqt[:], ps[:], ACTF.Square,
                            accum_out=vss[:, n * 4 + mt:n * 4 + mt + 1])
                        nc.sync.dma_start(
                            vN_s[n * 512 + mt * 128:n * 512 + (mt + 1) * 128,
                                 :], ev[:])
                    # -- gate logits: contraction-sharded over (x+c) quarter
                    ct = ctpool.tile([128, HPC, 512], F16, tag="ct")
                    nc.sync.dma_start(
                        ct[:], xcT[:, tok].rearrange("(kt p) m -> p kt m",
                                                     p=128))
                    gtps = ps_small([H, 512])
                    for kk in range(HPC):
                        nc.tensor.matmul(gtps[:H, :], wgt_sb[:, kk, :],
                                         ct[:, kk, :], start=(kk == 0),
                                         stop=(kk == HPC - 1))
                    gstg = spool.tile([H, 512], F32, tag="gstg", bufs=2)
                    nc.vector.tensor_copy(gstg[:], gtps[:H, :])
                    nc.scalar.dma_start(gt_in[half][:, ltok], gstg[:])
                    # -- deferred sumsq matmuls (inputs long since evacuated)
                    for pi in range(2):
                        ssps = ps_small([1, 512])
                        for m in range(4):
                            nc.tensor.matmul(ssps[:1, :], ones16,
                                             sqs[pi][m][:], start=(m == 0),
                                             stop=(m == 3))
                        ssev = spool.tile([1, 512], F32, tag="ssev", bufs=2)
                        nc.vector.tensor_copy(ssev[:], ssps[:1, :])
                        nc.sync.dma_start(ss_in[half][pi:pi + 1, ltok],
                                          ssev[:])
                # v sumsq for this half: transpose [128,16] -> row 2
                vssT = ps_small([128, 128])
                nc.tensor.matmul(vssT[:16, :],
                                 vss[:, half * 16:(half + 1) * 16], ident32,
                                 is_transpose=True)
                vssev = spool.tile([16, 128], F32, tag="vssev", bufs=2)
                nc.vector.tensor_copy(vssev[:], vssT[:16, :])
                nc.sync.dma_start(
                    ss_in[half][2:3, :].rearrange("a (b c) -> (a b) c", c=128),
                    vssev[:])
                # collectives for this half (overlap with the next half's P1)
                nc.gpsimd.collective_compute(
                    "AllReduce", ALU.add,
                    replica_groups=[[0, 1, 2, 3], [4, 5, 6, 7]],
                    ins=[ss_in[half].opt()], outs=[ss_out[half].opt()],
                )
                nc.gpsimd.collective_compute(
                    "ReduceScatter", ALU.add,
                    replica_groups=[[0, 1, 2, 3], [4, 5, 6, 7]],
                    ins=[gt_in[half].opt()], outs=[gt_out[half].opt()],
                )

            # =========================================================
            # P2 (per half): norm scales + gate decays
            # =========================================================
            ssgt = ppool.tile([128, NBLK, 7], F32, tag="ssgt")
            rsn = ppool.tile([128, NBLK, 3], F32, tag="rsn")
            skv = ppool.tile([128, NBLK], F32, tag="skv")
            gtd = ppool.tile([128, NBLK, HPC], F32R, tag="gtd")
            gtn = ppool.tile([128, NBLK, HPC], F32, tag="gtn")
            rf = ppool.tile([128, NCH, 2, HPC], F32, tag="rf")      # rowfac
            vf = ppool.tile([128, NCH, 2, HPC], F32, tag="vf")      # vfac
            eS = ppool.tile([128, NCH, HPC], F32, tag="eS")
            rf2 = ppool.tile([128, NCH, 2, HPC], F32, tag="rf2")

            def es_part(ch):
                # eS[ch] couples chunk ch and ch+1 (blocks 2ch+1, 2ch+2)
                b1 = 2 * ch + 1
                pt = ps_small([128, HPC])
                nc.tensor.matmul(pt[:], Om, gtd[:, b1, :], start=True,
                                 stop=False)
                nc.tensor.matmul(pt[:], Om, gtd[:, b1 + 1, :], start=False,
                                 stop=True)
                nc.scalar.activation(eS[:, ch, :], pt[:], ACTF.Exp)

            def p2_half(half):
                hb = slice(half * 16, (half + 1) * 16)
                for nn_ in range(4):
                    ltok = slice(nn_ * 512, (nn_ + 1) * 512)
                    srt = spool.tile([7, 512], F32, tag="srt", bufs=2)
                    nc.sync.dma_start(srt[0:3, :], ss_out[half][:, ltok])
                    nc.sync.dma_start(srt[3:7, :], gt_out[half][:, ltok])
                    for j in range(4):
                        tp = ps_small([128, 8])
                        nc.tensor.matmul(tp[:, :7],
                                         srt[:, j * 128:(j + 1) * 128],
                                         ident32[:7, :7], is_transpose=True)
                        nc.vector.tensor_copy(
                            ssgt[:, half * 16 + nn_ * 4 + j, :], tp[:, :7])
                nc.vector.tensor_scalar(rsn[:, hb], ssgt[:, hb, 0:3],
                                        1.0 / DIM, EPS, ALU.mult, ALU.add)
                nc.scalar.activation(rsn[:, hb], rsn[:, hb], ACTF.Ln)
                nc.scalar.activation(rsn[:, hb], rsn[:, hb], ACTF.Exp,
                                     scale=-0.5)
                nc.vector.tensor_mul(skv[:, hb], rsn[:, hb, 1], rsn[:, hb, 2])
                nc.scalar.activation(gtn[:, hb], ssgt[:, hb, 3:7], ACTF.Exp,
                                     scale=-1.0)
                nc.scalar.activation(gtd[:, hb], gtn[:, hb], ACTF.Ln, bias=1.0)
                if half == 1:
                    es_part(7)  # needs block 16 (half 1), deferred to here
                for ch in range(half * 8, half * 8 + 8):
                    b0, b1 = 2 * ch, 2 * ch + 1
                    p0 = ps_small([128, HPC])
                    nc.tensor.matmul(p0[:], Lc, gtd[:, b0, :], start=True,
                                     stop=True)
                    p1 = ps_small([128, HPC])
                    nc.tensor.matmul(p1[:], Lm, gtd[:, b1, :], start=True,
                                     stop=True)
                    if ch < NCH - 1 and ch != 7:
                        es_part(ch)
                    for blk01, bps in ((0, p0), (1, p1)):
                        blk = 2 * ch + blk01
                        # rowfac = exp(b') * sq * scale / VSH
                        nc.scalar.activation(rf[:, ch, blk01, :], bps[:],
                                             ACTF.Exp)
                        nc.vector.tensor_scalar(
                            rf[:, ch, blk01, :], rf[:, ch, blk01, :],
                            rsn[:, blk, 0:1], SCALE / VSH, ALU.mult, ALU.mult)
                        # vfac = exp(-b') * sk * sv * VSH
                        nc.scalar.activation(vf[:, ch, blk01, :], bps[:],
                                             ACTF.Exp, scale=-1.0)
                        nc.vector.tensor_scalar(
                            vf[:, ch, blk01, :], vf[:, ch, blk01, :],
                            skv[:, blk:blk + 1], VSH, ALU.mult, ALU.mult)
                # rf2 = rf^2 / HD (for the fused norm scale)
                nc.vector.scalar_tensor_tensor(
                    rf2[:, half * 8:(half + 1) * 8],
                    rf[:, half * 8:(half + 1) * 8],
                    1.0 / HD, rf[:, half * 8:(half + 1) * 8],
                    op0=ALU.mult, op1=ALU.mult)

            # =========================================================
            # P3: retention + fused norm/gate + out-proj, per chunk
            # =========================================================
            if int(os.environ.get("GR_BARRIER", "0")):
                tc.prologue_barrier()
            wo_sb = wpool.tile([128, HPC, DIM], F16, tag="wo")
            nc.sync.dma_start(wo_sb[:], wo.rearrange("(h p) m -> p h m", p=128))

            S_box = [None]

            def p3_chunk(ch):
                S_prev = S_box[0]
                tok = slice(ch * CS, (ch + 1) * CS)
                qc = rpool.tile([128, HPC, CS], F16, tag="qc")
                kc = rpool.tile([128, HPC, CS], F16, tag="kc")
                for t_, s_ in ((qc, qT_s), (kc, kT_s)):
                    nc.sync.dma_start(
                        t_[:], s_[:, tok].rearrange("(h p) m -> p h m", p=128))
                # k natural layout: one batched XBAR transpose per token block
                kn = []
                if ch < NCH - 1:
                    if KNPE:
                        for hl in range(HPC):
                            for ci in range(2):
                                tpk = ps_trp()
                                nc.tensor.transpose(
                                    tpk[:], kc[:, hl,
                                               ci * 128:ci * 128 + 128], i16)
                                knt = rpool.tile([128, 128], F16,
                                                 tag=f"kn{hl * 2 + ci}")
                                if (hl + ci) % 2 == 0:
                                    nc.scalar.copy(knt[:], tpk[:])
                                else:
                                    nc.vector.tensor_copy(knt[:], tpk[:])
                                kn.append(knt)
                    else:
                        for ci in range(2):
                            bt = slice(ch * CS + ci * 128,
                                       ch * CS + ci * 128 + 128)
                            knb = rpool.tile([128, PCOLS], F16,
                                             tag=f"knb{ci}")
                            nc.scalar.dma_start_transpose(
                                knb[:], kT_s[:, bt])
                            kn.append(knb)
                vcn = []
                for ci in range(2):
                    bt = slice(ch * CS + ci * 128, ch * CS + ci * 128 + 128)
                    vt = rpool.tile([128, PCOLS], F16, tag="vcn", bufs=4)
                    nc.scalar.dma_start(vt[:], vN_s[bt, :])
                    vcn.append(vt)
                sg = rpool.tile([128, HPC, CS], F16, tag="sg")
                nc.sync.dma_start(
                    sg[:], gT_s[:, tok].rearrange("(h p) m -> p h m", p=128))
                # vv = v * vfac
                vvt = rpool.tile([128, 2, HPC, HD], F16, tag="vvt")
                for ci in range(2):
                    for hl in range(HPC):
                        nc.vector.tensor_scalar(
                            vvt[:, ci, hl, :],
                            vcn[ci][:, hl * 128:(hl + 1) * 128],
                            vf[:, ch, ci, hl:hl + 1], None, ALU.mult)
                # state update FIRST: the S recurrence is the only true
                # cross-chunk serial chain, so emit it ahead of everything
                # else; o_raw below still uses the previous chunk's state.
                if ch < NCH - 1:
                    sps = ps_small([128, HPC, HD])
                    for hl in range(HPC):
                        hsl = slice(hl * 128, (hl + 1) * 128)
                        kn0 = kn[hl * 2][:] if KNPE else kn[0][:, hsl]
                        kn1 = kn[hl * 2 + 1][:] if KNPE else kn[1][:, hsl]
                        nc.tensor.matmul(sps[:, hl, :], kn0,
                                         vvt[:, 0, hl, :], start=True,
                                         stop=False)
                        nc.tensor.matmul(sps[:, hl, :], kn1,
                                         vvt[:, 1, hl, :], start=False,
                                         stop=True)
                    eSb = eS[:, ch, :].unsqueeze(2).to_broadcast(
                        [128, HPC, HD])
                    S_cur = rpool.tile([128, HPC, HD], F16, tag="S")
                    if ch > 0:
                        stmp = rpool.tile([128, HPC, HD], F32, tag="stmp")
                        nc.vector.tensor_tensor(stmp[:], sps[:], S_prev[:],
                                                ALU.add)
                        nc.vector.tensor_tensor(S_cur[:], stmp[:], eSb,
                                                ALU.mult)
                    else:
                        nc.vector.tensor_tensor(S_cur[:], sps[:], eSb,
                                                ALU.mult)
                    S_box[0] = S_cur
                # AT (masked): rows tj, cols ti
                at0s, at1s = [], []
                for hl in range(HPC):
                    at0ps = ps_small([128, 256])
                    nc.tensor.matmul(at0ps[:], kc[:, hl, 0:128], qc[:, hl, :],
                                     start=True, stop=True)
                    at0 = rpool.tile([128, CS], F16, tag="at0", bufs=4)
                    nc.vector.scalar_tensor_tensor(
                        at0[:], at0ps[:], 1.0, M0, op0=ALU.mult, op1=ALU.mult)
                    at0s.append(at0)
                    at1ps = ps_small([128, 128])
                    nc.tensor.matmul(at1ps[:], kc[:, hl, 128:256],
                                     qc[:, hl, 128:256], start=True, stop=True)
                    at1 = rpool.tile([128, 128], F16, tag="at1s", bufs=4)
                    nc.vector.scalar_tensor_tensor(
                        at1[:], at1ps[:], 1.0, Um32, op0=ALU.mult,
                        op1=ALU.mult)
                    at1s.append(at1)
                # o_raw = intra + inter; one PSUM tile per token half-block
                orp = []
                for ci in range(2):
                    orps = ps_oraw()
                    for hl in range(HPC):
                        reg = orps[:, hl, :]
                        mms = [(at0s[hl][:, ci * 128:ci * 128 + 128],
                                vvt[:, 0, hl, :])]
                        if ci == 1:
                            mms.append((at1s[hl][:], vvt[:, 1, hl, :]))
                        if ch > 0:
                            mms.append((qc[:, hl, ci * 128:ci * 128 + 128],
                                        S_prev[:, hl, :]))
                        for i, (lh, rh) in enumerate(mms):
                            nc.tensor.matmul(reg, lh, rh, start=(i == 0),
                                             stop=(i == len(mms) - 1))
                    orp.append(orps)
                # fused subln norm + rowfac: f = rf*rsqrt(rf^2*ss/HD + eps)
                ssum = rpool.tile([128, 2 * HPC], F32, tag="ssum")
                for idx in range(2 * HPC):
                    ci, hl = idx // HPC, idx % HPC
                    osq = rpool.tile([128, HD], F32, tag="osq", bufs=4)
                    nc.scalar.activation(osq[:], orp[ci][:, hl, :],
                                         ACTF.Square,
                                         accum_out=ssum[:, idx:idx + 1])
                rfv = rf[:, ch].rearrange("p a b -> p (a b)")
                rf2v = rf2[:, ch].rearrange("p a b -> p (a b)")
                dd = rpool.tile([128, 2 * HPC], F32, tag="dd")
                nc.vector.tensor_tensor(dd[:], rf2v, ssum[:], ALU.mult)
                nc.scalar.activation(dd[:], dd[:], ACTF.Ln, bias=epsb)
                nc.scalar.activation(dd[:], dd[:], ACTF.Exp, scale=-0.5)
                ff = rpool.tile([128, 2 * HPC], F32, tag="ff")
                nc.vector.tensor_tensor(ff[:], rfv, dd[:], ALU.mult)
                o_n = rpool.tile([128, 2 * HPC, HD], F16, tag="o_n")
                for ci in range(2):
                    hsl = slice(ci * HPC, (ci + 1) * HPC)
                    nc.vector.tensor_tensor(
                        o_n[:, hsl, :], orp[ci][:],
                        ff[:, hsl].unsqueeze(2).to_broadcast([128, HPC, HD]),
                        ALU.mult)
                # transpose to [chan, tok] + gate (idx = ci*HPC + hl)
                go = []
                for ci in range(2):
                    for hl in range(HPC):
                        idx = ci * HPC + hl
                        got = rpool.tile([128, 128], F16, tag=f"go{idx}")
                        if PET:
                            trp = ps_trp()
                            nc.tensor.transpose(trp[:], o_n[:, idx, :], i16)
                            nc.vector.tensor_mul(
                                got[:], trp[:],
                                sg[:, hl, ci * 128:ci * 128 + 128])
                        else:
                            tro = rpool.tile([128, 128], F16, tag=f"tr{idx}")
                            nc.sync.dma_start_transpose(
                                tro[:], o_n[:, idx, :])
                            nc.vector.tensor_mul(
                                got[:], tro[:],
                                sg[:, hl, ci * 128:ci * 128 + 128])
                        go.append(got)
                # out-proj for this chunk's two token tiles
                for m01 in range(2):
                    for nb in range(DIM // 512):
                        ps = ps_big()
                        nsl = slice(nb * 512, (nb + 1) * 512)
                        for hl in range(HPC):
                            nc.tensor.matmul(ps[:], go[m01 * HPC + hl][:],
                                             wo_sb[:, hl, nsl],
                                             start=(hl == 0),
                                             stop=(hl == HPC - 1))
                        oo = epool.tile([128, 512], F16, tag="oo", bufs=4)
                        if nb % 2 == 0:
                            nc.vector.tensor_copy(oo[:], ps[:])
                            nc.sync.dma_start(
                                out[ch * CS + m01 * 128:
                                    ch * CS + m01 * 128 + 128, nsl], oo[:])
                        else:
                            nc.scalar.copy(oo[:], ps[:])
                            nc.scalar.dma_start(
                                out[ch * CS + m01 * 128:
                                    ch * CS + m01 * 128 + 128, nsl], oo[:])

            # emission order: P2 half0 right after P1 (its collective
            # completed mid-P1), first two chunks, then P2 half1 (its
            # collective completes around P1 end), then the rest.
            p2_half(0)
            p3_chunk(0)
            p3_chunk(1)
            p2_half(1)
            for ch in range(2, NCH):
                p3_chunk(ch)

            if debug and DEBUG_LVL >= 3:
                nc.sync.dma_start(
                    dbg("dbg_rf", [128, NCH * 2 * HPC]),
                    rf[:].rearrange("p a b c -> p (a b c)"))
                nc.sync.dma_start(
                    dbg("dbg_vf", [128, NCH * 2 * HPC]),
                    vf[:].rearrange("p a b c -> p (a b c)"))
                nc.sync.dma_start(
                    dbg("dbg_eS", [128, NCH * HPC]),
                    eS[:].rearrange("p a b -> p (a b)"))
            if debug and DEBUG_LVL >= 2:
                nc.sync.dma_start(
                    dbg("dbg_rsn", [128, NBLK * 3]),
                    rsn[:].rearrange("p a b -> p (a b)"))
                nc.sync.dma_start(
                    dbg("dbg_gtd", [128, NBLK * HPC]),
                    gtd[:].bitcast(F32).rearrange("p a b -> p (a b)"))

    nc.compile()
    return nc


def _prep_inputs(x, c, Wq, Wk, Wv, Wg, Wgt, Wo):
    """Build the 8 per-core input maps (host-side sharding / layout)."""
    consts = np.ascontiguousarray(_consts_np())
    c16 = np.concatenate(
        [np.eye(128, dtype=np.float16), np.ones((128, 1), np.float16)], axis=1)
    in_maps = []
    xTs = [np.ascontiguousarray(x[b].T.astype(np.float16)) for b in range(B)]
    xc = x + c
    xcTs = [np.ascontiguousarray(xc[b].T.astype(np.float16)) for b in range(B)]
    for core in range(NCORE):
        b, g = core // 4, core % 4
        cols = slice(g * PCOLS, (g + 1) * PCOLS)
        in_maps.append({
            "xT": xTs[b],
            "xcT": np.ascontiguousarray(xcTs[b][cols, :]),
            "wq": np.ascontiguousarray(Wq[:, cols]).astype(np.float16),
            "wk": np.ascontiguousarray(Wk[:, cols]).astype(np.float16),
            "wv": np.ascontiguousarray(Wv[:, cols]).astype(np.float16),
            "wg": np.ascontiguousarray(Wg[:, cols]).astype(np.float16),
            "wgt": np.ascontiguousarray(Wgt[cols, :]).astype(np.float16),
            "wo": np.ascontiguousarray(Wo[cols, :]).astype(np.float16),
            "consts": consts,
            "c16": c16,
        })
    return in_maps


def kernel(x, c, Wq, Wk, Wv, Wg, Wgt, Wo, _want_results=False):
    key = "nc_dbg" if DEBUG else "nc"
    if key not in _cache:
        _cache[key] = build(debug=DEBUG)
    nc = _cache[key]
    in_maps = _prep_inputs(np.asarray(x, np.float32), np.asarray(c, np.float32),
                           np.asarray(Wq, np.float32), np.asarray(Wk, np.float32),
                           np.asarray(Wv, np.float32), np.asarray(Wg, np.float32),
                           np.asarray(Wgt, np.float32), np.asarray(Wo, np.float32))
    res = bass_utils.run_bass_kernel_spmd(
        nc, in_maps, core_ids=list(range(NCORE)), trace=TRACE)
    out = np.zeros((B, T, DIM), np.float32)
    for core in range(NCORE):
        out[core // 4] += res.results[core]["out"].astype(np.float32)
    if _want_results:
        return out, res
    return out


# revision 30
# speedup vs baseline: 1.4317x; 1.0229x over previous
"""GateRetention Trainium2 kernel (Bass/Tile), 8-core tensor-parallel.

Sharding: core grid (batch b = core//4, head-group g = core%4); each core owns
4 heads (512 cols of the q/k/v/g projections, 512 rows of Wo) of one batch.
RMS-norm statistics are AllReduced across each batch's 4 cores; gate logits
are contraction-sharded (host pre-adds x+c, each core contracts a 512-row
quarter for all 16 heads) and ReduceScattered so each core gets its 4 heads.
Collectives are split per half-T and issued mid-P1 so they overlap compute.
Out-proj partials are summed on the host (row-parallel TP gather).

Precision: all projections and retention in fp16 with fp32 PSUM accumulation;
a 2^-2 exponent shift on vfac keeps decayed v tiles in fp16 range.  The
rowfac (per-token decay * q-norm * scale) is folded into the subln norm
scale f = rf * rsqrt(rf^2 * sumsq/HD + eps), so the raw retention output is
normalized+scaled in one pass (exactly equal to norm(rf*o_raw)).

kernel(**inputs) takes the FULL inputs from reference.setup_inputs() and
returns the FULL [B, T, DIM] fp32 output.
"""
import os
import sys

sys.path.insert(0, "/opt/trn_rl_repo")

import numpy as np

import concourse.bass as bass
import concourse.bacc as bacc
import concourse.tile as tile
import concourse.mybir as mybir
from concourse import bass_utils

F32 = mybir.dt.float32
F32R = mybir.dt.float32r
F16 = mybir.dt.float16
AX = mybir.AxisListType
ALU = mybir.AluOpType
ACTF = mybir.ActivationFunctionType

B, T, DIM = 2, 4096, 2048
H, HD = 16, 128
CS = 256
NCH = T // CS              # 16 chunks
EPS = 1e-5
GLN = 16.0
SCALE = HD ** -0.5
NCORE = 8
HPC = 4                    # heads per core
PCOLS = HPC * HD           # 512 cols per core
NBLK = T // 128            # 32 token blocks of 128
NT = T // 512              # 8 token n-tiles
TH = T // 2                # tokens per collective half
VSH = 2.0 ** -2            # fp16 range shift on vv; inverse folded into rowfac

DEBUG_LVL = int(os.environ.get("GR_DEBUG", "0"))
DEBUG = bool(DEBUG_LVL)
TRACE = bool(int(os.environ.get("GR_TRACE", "0")))
PET = not bool(int(os.environ.get("GR_XPT", "0")))  # PE transpose for o_n
KNPE = bool(int(os.environ.get("GR_KNPE", "0")))    # PE-transpose fallback for kn

_cache = {}


def _consts_np():
    """[128, 904] fp32: identity | Lm | Om | Um | ones8 | Lc | M0."""
    ident = np.eye(128, dtype=np.float32)
    jj, ii = np.meshgrid(np.arange(128), np.arange(128), indexing="ij")
    Lm = np.where(jj <= ii, -1.0 / GLN, 0.0).astype(np.float32)
    Om = np.full((128, 128), -1.0 / GLN, np.float32)
    Um = np.where(jj <= ii, 1.0, 0.0).astype(np.float32)
    ones = np.ones((128, 8), np.float32)
    ones[:, 1] = EPS                       # col 513: eps bias for Ln
    # Lc: b_i - b_mid for block0 = +1/GLN * sum_{j>i} sp_j
    Lc = np.where(jj > ii, 1.0 / GLN, 0.0).astype(np.float32)
    M0 = np.concatenate([Um, np.ones((128, 128), np.float32)], axis=1)
    return np.concatenate([ident, Lm, Om, Um, ones, Lc, M0], axis=1)


def build(debug=False):
    nc = bacc.Bacc("TRN2", target_bir_lowering=False, debug=False,
                   enable_asserts=False, num_devices=NCORE)

    # ---------------- I/O ----------------
    xT = nc.dram_tensor("xT", [DIM, T], F16, kind="ExternalInput").ap()
    # (x+c)^T row-quarter for this core's contraction shard of the gate logits
    xcT = nc.dram_tensor("xcT", [PCOLS, T], F16, kind="ExternalInput").ap()
    wq = nc.dram_tensor("wq", [DIM, PCOLS], F16, kind="ExternalInput").ap()
    wk = nc.dram_tensor("wk", [DIM, PCOLS], F16, kind="ExternalInput").ap()
    wv = nc.dram_tensor("wv", [DIM, PCOLS], F16, kind="ExternalInput").ap()
    wg = nc.dram_tensor("wg", [DIM, PCOLS], F16, kind="ExternalInput").ap()
    wgt = nc.dram_tensor("wgt", [PCOLS, H], F16, kind="ExternalInput").ap()
    wo = nc.dram_tensor("wo", [PCOLS, DIM], F16, kind="ExternalInput").ap()
    consts = nc.dram_tensor("consts", [128, 904], F32R, kind="ExternalInput").ap()
    c16 = nc.dram_tensor("c16", [128, 129], F16, kind="ExternalInput").ap()
    out = nc.dram_tensor("out", [T, DIM], F16, kind="ExternalOutput").ap()

    def dbg(name, shape, dtype=F32):
        return nc.dram_tensor(name, shape, dtype, kind="ExternalOutput").ap()

    with tile.TileContext(nc) as tc:
        with (
            tc.tile_pool(name="const", bufs=1) as cpool,
            tc.tile_pool(name="wts", bufs=1) as wpool,
            tc.tile_pool(name="xstream", bufs=4) as xpool,
            tc.tile_pool(name="cstream", bufs=2) as ctpool,
            tc.tile_pool(name="evac", bufs=4) as epool,
            tc.tile_pool(name="persist", bufs=1) as ppool,
            tc.tile_pool(name="small", bufs=2) as spool,
            tc.tile_pool(name="ret", bufs=2) as rpool,
            tc.tile_pool(name="ps", bufs=1, space="PSUM") as psp,
            tc.tile_pool(name="dram", bufs=1, space="DRAM") as dpool,
        ):
            def ps_big():
                return psp.tile([128, 512], F32, tag="big", bufs=2, name="psbig")

            def ps_small(shape=None, dtype=F32):
                return psp.tile(shape or [128, 256], dtype, tag="small", bufs=2,
                                name="pssmall")

            def ps_oraw():
                return psp.tile([128, HPC, HD], F32, tag="oraw", bufs=2,
                                name="psoraw")

            def ps_trp():
                return psp.tile([128, 128], F16, tag="trp", bufs=2,
                                name="pstrp")

            # ---------------- constants ----------------
            cst = cpool.tile([128, 904], F32R, tag="consts")
            nc.sync.dma_start(cst[:], consts)
            ident32 = cst[:, 0:128].bitcast(F32)
            Lm = cst[:, 128:256]
            Om = cst[:, 256:384]
            Um32 = cst[:, 384:512].bitcast(F32)
            epsb = cst[:, 513:514].bitcast(F32)
            Lc = cst[:, 520:648]
            M0 = cst[:, 648:904].bitcast(F32)
            aux16 = cpool.tile([128, 129], F16, tag="i16")
            nc.sync.dma_start(aux16[:], c16)
            i16 = aux16[:, 0:128]
            ones16 = aux16[:, 128:129]

            # ---------------- DRAM scratch (fp16) ----------------
            if debug:
                qT_s = dbg("dbg_qT", [PCOLS, T], F16)
                kT_s = dbg("dbg_kT", [PCOLS, T], F16)
                vN_s = dbg("dbg_vN", [T, PCOLS], F16)
                gT_s = dbg("dbg_gT", [PCOLS, T], F16)
            else:
                qT_s = nc.dram_tensor("qT_s", [PCOLS, T], F16,
                                      kind="Internal").ap()
                kT_s = nc.dram_tensor("kT_s", [PCOLS, T], F16,
                                      kind="Internal").ap()
                vN_s = nc.dram_tensor("vN_s", [T, PCOLS], F16,
                                      kind="Internal").ap()
                gT_s = nc.dram_tensor("gT_s", [PCOLS, T], F16,
                                      kind="Internal").ap()
            ss_in = [nc.dram_tensor(f"ss_in{h}", [3, TH], F32,
                                    kind="Internal").ap() for h in range(2)]
            ss_out = [nc.dram_tensor(f"ss_out{h}", [3, TH], F32,
                                     kind="Internal").ap() for h in range(2)]
            gt_in = [nc.dram_tensor(f"gt_in{h}", [H, TH], F32,
                                    kind="Internal").ap() for h in range(2)]
            gt_out = [nc.dram_tensor(f"gt_out{h}", [HPC, TH], F32,
                                     kind="Internal").ap() for h in range(2)]

            # =========================================================
            # P1: single fused pass: q,k,g (T-layout), v (natural),
            # gate-logit partials; all fp16 math, fp32 PSUM.
            # Collectives per half-T, issued mid-stream so they overlap.
            # =========================================================
            vss = ppool.tile([128, NBLK], F32, tag="vss")

            def load_w(wdram, tag):
                wt = wpool.tile([128, 16, 512], F16, tag=tag)
                nc.sync.dma_start(
                    wt[:], wdram.rearrange("(kt p) m -> p kt m", p=128))
                return wt

            wq_sb = load_w(wq, "w0")
            wk_sb = load_w(wk, "w1")
            wv_sb = load_w(wv, "w2")
            wg_sb = load_w(wg, "w3")
            wgt_sb = wpool.tile([128, HPC, H], F16, tag="wgt")
            nc.sync.dma_start(wgt_sb[:],
                              wgt.rearrange("(kt p) m -> p kt m", p=128))

            I32 = mybir.dt.int32

            def newton_rsqrt(out, din, tmp):
                """out = rsqrt(din) on the vector engine only (no ACT table
                loads): bit-trick seed + 2 Newton iterations (~1e-6 rel)."""
                oi = out.bitcast(I32)
                di = din.bitcast(I32)
                nc.vector.tensor_scalar(oi, di, 1, None,
                                        ALU.logical_shift_right)
                nc.vector.tensor_scalar(oi, oi, -1, 0x5f3759df, ALU.mult,
                                        ALU.add)
                for _ in range(2):
                    nc.vector.tensor_tensor(tmp, out, out, ALU.mult)
                    nc.vector.tensor_tensor(tmp, tmp, din, ALU.mult)
                    nc.vector.tensor_scalar(tmp, tmp, -0.5, 1.5, ALU.mult,
                                            ALU.add)
                    nc.vector.tensor_tensor(out, out, tmp, ALU.mult)

            def tproj_mms(ps, wt, xth, m):
                for k in range(16):
                    nc.tensor.matmul(
                        ps[:], wt[:, k, m * 128:(m + 1) * 128],
                        xth[k // 8][:, k % 8, :], start=(k == 0), stop=(k == 15))

            for half in range(2):
                for n in range(half * 4, half * 4 + 4):
                    tok = slice(n * 512, (n + 1) * 512)
                    ltok = slice(n * 512 - half * TH, (n + 1) * 512 - half * TH)
                    xth = []
                    for h2 in range(2):
                        xt = xpool.tile([128, 8, 512], F16, tag="xt")
                        nc.sync.dma_start(
                            xt[:], xT[h2 * 1024:(h2 + 1) * 1024, tok].rearrange(
                                "(kt p) m -> p kt m", p=128))
                        xth.append(xt)
                    # -- q, k: T-layout; squares kept for deferred sumsq
                    sqs = {0: [], 1: []}
                    for pi, (wsb, sdram) in enumerate(((wq_sb, qT_s),
                                                       (wk_sb, kT_s))):
                        for m in range(4):
                            ps = ps_big()
                            tproj_mms(ps, wsb, xth, m)
                            ev = epool.tile([128, 512], F16, tag="ev")
                            sqt = epool.tile([128, 512], F16, tag="sq",
                                             bufs=10)
                            if m % 2 == 0:
                                nc.vector.tensor_copy(ev[:], ps[:])
                                nc.scalar.activation(sqt[:], ps[:],
                                                     ACTF.Square)
                                nc.sync.dma_start(
                                    sdram[m * 128:(m + 1) * 128, tok], ev[:])
                            else:
                                nc.scalar.copy(ev[:], ps[:])
                                nc.vector.tensor_tensor(sqt[:], ev[:], ev[:],
                                                        ALU.mult)
                                nc.scalar.dma_start(
                                    sdram[m * 128:(m + 1) * 128, tok], ev[:])
                            sqs[pi].append(sqt)
                    # -- g: silu, T-layout
                    for m in range(4):
                        ps = ps_big()
                        tproj_mms(ps, wg_sb, xth, m)
                        ev = epool.tile([128, 512], F16, tag="ev")
                        nc.scalar.activation(ev[:], ps[:], ACTF.Silu)
                        nc.scalar.dma_start(gT_s[m * 128:(m + 1) * 128, tok],
                                            ev[:])
                    # -- v: natural layout + accumulated sumsq
                    for mt in range(4):
                        msl = slice(mt * 128, (mt + 1) * 128)
                        ps = ps_big()
                        for k in range(16):
                            nc.tensor.matmul(
                                ps[:], xth[k // 8][:, k % 8, msl],
                                wv_sb[:, k, :], start=(k == 0), stop=(k == 15))
                        ev = epool.tile([128, 512], F16, tag="ev")
                        sqt = epool.tile([128, 512], F16, tag="vsq", bufs=2)
                        nc.vector.tensor_copy(ev[:], ps[:])
                        nc.scalar.activation(
                            sqt[:], ps[:], ACTF.Square,
                            accum_out=vss[:, n * 4 + mt:n * 4 + mt + 1])
                        nc.sync.dma_start(
                            vN_s[n * 512 + mt * 128:n * 512 + (mt + 1) * 128,
                                 :], ev[:])
                    # -- gate logits: contraction-sharded over (x+c) quarter
                    ct = ctpool.tile([128, HPC, 512], F16, tag="ct")
                    nc.sync.dma_start(
                        ct[:], xcT[:, tok].rearrange("(kt p) m -> p kt m",
                                                     p=128))
                    gtps = ps_small([H, 512])
                    for kk in range(HPC):
                        nc.tensor.matmul(gtps[:H, :], wgt_sb[:, kk, :],
                                         ct[:, kk, :], start=(kk == 0),
                                         stop=(kk == HPC - 1))
                    gstg = spool.tile([H, 512], F32, tag="gstg", bufs=2)
                    nc.vector.tensor_copy(gstg[:], gtps[:H, :])
                    nc.scalar.dma_start(gt_in[half][:, ltok], gstg[:])
                    # -- deferred sumsq matmuls (inputs long since evacuated)
                    for pi in range(2):
                        ssps = ps_small([1, 512])
                        for m in range(4):
                            nc.tensor.matmul(ssps[:1, :], ones16,
                                             sqs[pi][m][:], start=(m == 0),
                                             stop=(m == 3))
                        ssev = spool.tile([1, 512], F32, tag="ssev", bufs=2)
                        nc.vector.tensor_copy(ssev[:], ssps[:1, :])
                        nc.sync.dma_start(ss_in[half][pi:pi + 1, ltok],
                                          ssev[:])
                # v sumsq for this half: transpose [128,16] -> row 2
                vssT = ps_small([128, 128])
                nc.tensor.matmul(vssT[:16, :],
                                 vss[:, half * 16:(half + 1) * 16], ident32,
                                 is_transpose=True)
                vssev = spool.tile([16, 128], F32, tag="vssev", bufs=2)
                nc.vector.tensor_copy(vssev[:], vssT[:16, :])
                nc.sync.dma_start(
                    ss_in[half][2:3, :].rearrange("a (b c) -> (a b) c", c=128),
                    vssev[:])
                # collectives for this half (overlap with the next half's P1)
                nc.gpsimd.collective_compute(
                    "AllReduce", ALU.add,
                    replica_groups=[[0, 1, 2, 3], [4, 5, 6, 7]],
                    ins=[ss_in[half].opt()], outs=[ss_out[half].opt()],
                )
                nc.gpsimd.collective_compute(
                    "ReduceScatter", ALU.add,
                    replica_groups=[[0, 1, 2, 3], [4, 5, 6, 7]],
                    ins=[gt_in[half].opt()], outs=[gt_out[half].opt()],
                )

            # =========================================================
            # P2 (per half): norm scales + gate decays
            # =========================================================
            ssgt = ppool.tile([128, NBLK, 7], F32, tag="ssgt")
            rsn = ppool.tile([128, NBLK, 3], F32, tag="rsn")
            skv = ppool.tile([128, NBLK], F32, tag="skv")
            gtd = ppool.tile([128, NBLK, HPC], F32R, tag="gtd")
            gtn = ppool.tile([128, NBLK, HPC], F32, tag="gtn")
            rf = ppool.tile([128, NCH, 2, HPC], F32, tag="rf")      # rowfac
            vf = ppool.tile([128, NCH, 2, HPC], F32, tag="vf")      # vfac
            eS = ppool.tile([128, NCH, HPC], F32, tag="eS")
            rf2 = ppool.tile([128, NCH, 2, HPC], F32, tag="rf2")

            def es_part(ch):
                # eS[ch] couples chunk ch and ch+1 (blocks 2ch+1, 2ch+2)
                b1 = 2 * ch + 1
                pt = ps_small([128, HPC])
                nc.tensor.matmul(pt[:], Om, gtd[:, b1, :], start=True,
                                 stop=False)
                nc.tensor.matmul(pt[:], Om, gtd[:, b1 + 1, :], start=False,
                                 stop=True)
                nc.scalar.activation(eS[:, ch, :], pt[:], ACTF.Exp)

            def p2_half(half):
                hb = slice(half * 16, (half + 1) * 16)
                for nn_ in range(4):
                    ltok = slice(nn_ * 512, (nn_ + 1) * 512)
                    srt = spool.tile([7, 512], F32, tag="srt", bufs=2)
                    nc.sync.dma_start(srt[0:3, :], ss_out[half][:, ltok])
                    nc.sync.dma_start(srt[3:7, :], gt_out[half][:, ltok])
                    for j in range(4):
                        tp = ps_small([128, 8])
                        nc.tensor.matmul(tp[:, :7],
                                         srt[:, j * 128:(j + 1) * 128],
                                         ident32[:7, :7], is_transpose=True)
                        nc.vector.tensor_copy(
                            ssgt[:, half * 16 + nn_ * 4 + j, :], tp[:, :7])
                rsw = spool.tile([128, 16, 3], F32, tag="rsw", bufs=2)
                rst = spool.tile([128, 16, 3], F32, tag="rst", bufs=2)
                nc.vector.tensor_scalar(rsw[:], ssgt[:, hb, 0:3],
                                        1.0 / DIM, EPS, ALU.mult, ALU.add)
                newton_rsqrt(rsn[:, hb], rsw[:], rst[:])
                nc.vector.tensor_mul(skv[:, hb], rsn[:, hb, 1], rsn[:, hb, 2])
                nc.scalar.activation(gtn[:, hb], ssgt[:, hb, 3:7], ACTF.Exp,
                                     scale=-1.0)
                nc.scalar.activation(gtd[:, hb], gtn[:, hb], ACTF.Ln, bias=1.0)
                if half == 1:
                    es_part(7)  # needs block 16 (half 1), deferred to here
                for ch in range(half * 8, half * 8 + 8):
                    b0, b1 = 2 * ch, 2 * ch + 1
                    p0 = ps_small([128, HPC])
                    nc.tensor.matmul(p0[:], Lc, gtd[:, b0, :], start=True,
                                     stop=True)
                    p1 = ps_small([128, HPC])
                    nc.tensor.matmul(p1[:], Lm, gtd[:, b1, :], start=True,
                                     stop=True)
                    if ch < NCH - 1 and ch != 7:
                        es_part(ch)
                    for blk01, bps in ((0, p0), (1, p1)):
                        blk = 2 * ch + blk01
                        # rowfac = exp(b') * sq * scale / VSH
                        nc.scalar.activation(rf[:, ch, blk01, :], bps[:],
                                             ACTF.Exp)
                        nc.vector.tensor_scalar(
                            rf[:, ch, blk01, :], rf[:, ch, blk01, :],
                            rsn[:, blk, 0:1], SCALE / VSH, ALU.mult, ALU.mult)
                        # vfac = exp(-b') * sk * sv * VSH
                        nc.scalar.activation(vf[:, ch, blk01, :], bps[:],
                                             ACTF.Exp, scale=-1.0)
                        nc.vector.tensor_scalar(
                            vf[:, ch, blk01, :], vf[:, ch, blk01, :],
                            skv[:, blk:blk + 1], VSH, ALU.mult, ALU.mult)
                # rf2 = rf^2 / HD (for the fused norm scale)
                nc.vector.scalar_tensor_tensor(
                    rf2[:, half * 8:(half + 1) * 8],
                    rf[:, half * 8:(half + 1) * 8],
                    1.0 / HD, rf[:, half * 8:(half + 1) * 8],
                    op0=ALU.mult, op1=ALU.mult)

            # =========================================================
            # P3: retention + fused norm/gate + out-proj, per chunk
            # =========================================================
            if int(os.environ.get("GR_BARRIER", "0")):
                tc.prologue_barrier()
            wo_sb = wpool.tile([128, HPC, DIM], F16, tag="wo")
            nc.sync.dma_start(wo_sb[:], wo.rearrange("(h p) m -> p h m", p=128))

            S_box = [None]

            def p3_chunk(ch):
                S_prev = S_box[0]
                tok = slice(ch * CS, (ch + 1) * CS)
                qc = rpool.tile([128, HPC, CS], F16, tag="qc")
                kc = rpool.tile([128, HPC, CS], F16, tag="kc")
                for t_, s_ in ((qc, qT_s), (kc, kT_s)):
                    nc.sync.dma_start(
                        t_[:], s_[:, tok].rearrange("(h p) m -> p h m", p=128))
                # k natural layout: one batched XBAR transpose per token block
                kn = []
                if ch < NCH - 1:
                    if KNPE:
                        for hl in range(HPC):
                            for ci in range(2):
                                tpk = ps_trp()
                                nc.tensor.transpose(
                                    tpk[:], kc[:, hl,
                                               ci * 128:ci * 128 + 128], i16)
                                knt = rpool.tile([128, 128], F16,
                                                 tag=f"kn{hl * 2 + ci}")
                                if (hl + ci) % 2 == 0:
                                    nc.scalar.copy(knt[:], tpk[:])
                                else:
                                    nc.vector.tensor_copy(knt[:], tpk[:])
                                kn.append(knt)
                    else:
                        for ci in range(2):
                            bt = slice(ch * CS + ci * 128,
                                       ch * CS + ci * 128 + 128)
                            knb = rpool.tile([128, PCOLS], F16,
                                             tag=f"knb{ci}")
                            nc.sync.dma_start_transpose(
                                knb[:], kT_s[:, bt])
                            kn.append(knb)
                vcn = []
                for ci in range(2):
                    bt = slice(ch * CS + ci * 128, ch * CS + ci * 128 + 128)
                    vt = rpool.tile([128, PCOLS], F16, tag="vcn", bufs=4)
                    nc.scalar.dma_start(vt[:], vN_s[bt, :])
                    vcn.append(vt)
                sg = rpool.tile([128, HPC, CS], F16, tag="sg")
                nc.scalar.dma_start(
                    sg[:], gT_s[:, tok].rearrange("(h p) m -> p h m", p=128))
                # vv = v * vfac
                vvt = rpool.tile([128, 2, HPC, HD], F16, tag="vvt")
                for ci in range(2):
                    for hl in range(HPC):
                        nc.vector.tensor_scalar(
                            vvt[:, ci, hl, :],
                            vcn[ci][:, hl * 128:(hl + 1) * 128],
                            vf[:, ch, ci, hl:hl + 1], None, ALU.mult)
                # state update FIRST: the S recurrence is the only true
                # cross-chunk serial chain, so emit it ahead of everything
                # else; o_raw below still uses the previous chunk's state.
                if ch < NCH - 1:
                    sps = ps_small([128, HPC, HD])
                    for hl in range(HPC):
                        hsl = slice(hl * 128, (hl + 1) * 128)
                        kn0 = kn[hl * 2][:] if KNPE else kn[0][:, hsl]
                        kn1 = kn[hl * 2 + 1][:] if KNPE else kn[1][:, hsl]
                        nc.tensor.matmul(sps[:, hl, :], kn0,
                                         vvt[:, 0, hl, :], start=True,
                                         stop=False)
                        nc.tensor.matmul(sps[:, hl, :], kn1,
                                         vvt[:, 1, hl, :], start=False,
                                         stop=True)
                    eSb = eS[:, ch, :].unsqueeze(2).to_broadcast(
                        [128, HPC, HD])
                    S_cur = rpool.tile([128, HPC, HD], F16, tag="S")
                    if ch > 0:
                        stmp = rpool.tile([128, HPC, HD], F32, tag="stmp")
                        nc.vector.tensor_tensor(stmp[:], sps[:], S_prev[:],
                                                ALU.add)
                        nc.vector.tensor_tensor(S_cur[:], stmp[:], eSb,
                                                ALU.mult)
                    else:
                        nc.vector.tensor_tensor(S_cur[:], sps[:], eSb,
                                                ALU.mult)
                    S_box[0] = S_cur
                # AT (masked): rows tj, cols ti
                at0s, at1s = [], []
                for hl in range(HPC):
                    at0ps = ps_small([128, 256])
                    nc.tensor.matmul(at0ps[:], kc[:, hl, 0:128], qc[:, hl, :],
                                     start=True, stop=True)
                    at0 = rpool.tile([128, CS], F16, tag="at0", bufs=4)
                    nc.vector.scalar_tensor_tensor(
                        at0[:], at0ps[:], 1.0, M0, op0=ALU.mult, op1=ALU.mult)
                    at0s.append(at0)
                    at1ps = ps_small([128, 128])
                    nc.tensor.matmul(at1ps[:], kc[:, hl, 128:256],
                                     qc[:, hl, 128:256], start=True, stop=True)
                    at1 = rpool.tile([128, 128], F16, tag="at1s", bufs=4)
                    nc.vector.scalar_tensor_tensor(
                        at1[:], at1ps[:], 1.0, Um32, op0=ALU.mult,
                        op1=ALU.mult)
                    at1s.append(at1)
                # o_raw = intra + inter; one PSUM tile per token half-block
                orp = []
                for ci in range(2):
                    orps = ps_oraw()
                    for hl in range(HPC):
                        reg = orps[:, hl, :]
                        mms = [(at0s[hl][:, ci * 128:ci * 128 + 128],
                                vvt[:, 0, hl, :])]
                        if ci == 1:
                            mms.append((at1s[hl][:], vvt[:, 1, hl, :]))
                        if ch > 0:
                            mms.append((qc[:, hl, ci * 128:ci * 128 + 128],
                                        S_prev[:, hl, :]))
                        for i, (lh, rh) in enumerate(mms):
                            nc.tensor.matmul(reg, lh, rh, start=(i == 0),
                                             stop=(i == len(mms) - 1))
                    orp.append(orps)
                # fused subln norm + rowfac: f = rf*rsqrt(rf^2*ss/HD + eps)
                ssum = rpool.tile([128, 2 * HPC], F32, tag="ssum")
                for idx in range(2 * HPC):
                    ci, hl = idx // HPC, idx % HPC
                    osq = rpool.tile([128, HD], F32, tag="osq", bufs=4)
                    nc.scalar.activation(osq[:], orp[ci][:, hl, :],
                                         ACTF.Square,
                                         accum_out=ssum[:, idx:idx + 1])
                rfv = rf[:, ch].rearrange("p a b -> p (a b)")
                rf2v = rf2[:, ch].rearrange("p a b -> p (a b)")
                dd = rpool.tile([128, 2 * HPC], F32, tag="dd")
                nc.vector.tensor_tensor(dd[:], rf2v, ssum[:], ALU.mult)
                nc.vector.tensor_scalar(dd[:], dd[:], EPS, None, ALU.add)
                yy = rpool.tile([128, 2 * HPC], F32, tag="yy")
                yt = rpool.tile([128, 2 * HPC], F32, tag="yt")
                newton_rsqrt(yy[:], dd[:], yt[:])
                ff = rpool.tile([128, 2 * HPC], F32, tag="ff")
                nc.vector.tensor_tensor(ff[:], rfv, yy[:], ALU.mult)
                o_n = rpool.tile([128, 2 * HPC, HD], F16, tag="o_n")
                for ci in range(2):
                    hsl = slice(ci * HPC, (ci + 1) * HPC)
                    nc.vector.tensor_tensor(
                        o_n[:, hsl, :], orp[ci][:],
                        ff[:, hsl].unsqueeze(2).to_broadcast([128, HPC, HD]),
                        ALU.mult)
                # transpose to [chan, tok] + gate (idx = ci*HPC + hl)
                go = []
                for ci in range(2):
                    for hl in range(HPC):
                        idx = ci * HPC + hl
                        got = rpool.tile([128, 128], F16, tag=f"go{idx}")
                        if PET:
                            trp = ps_trp()
                            nc.tensor.transpose(trp[:], o_n[:, idx, :], i16)
                            nc.vector.tensor_mul(
                                got[:], trp[:],
                                sg[:, hl, ci * 128:ci * 128 + 128])
                        else:
                            tro = rpool.tile([128, 128], F16, tag=f"tr{idx}")
                            nc.sync.dma_start_transpose(
                                tro[:], o_n[:, idx, :])
                            nc.vector.tensor_mul(
                                got[:], tro[:],
                                sg[:, hl, ci * 128:ci * 128 + 128])
                        go.append(got)
                # out-proj for this chunk's two token tiles
                for m01 in range(2):
                    for nb in range(DIM // 512):
                        ps = ps_big()
                        nsl = slice(nb * 512, (nb + 1) * 512)
                        for hl in range(HPC):
                            nc.tensor.matmul(ps[:], go[m01 * HPC + hl][:],
                                             wo_sb[:, hl, nsl],
                                             start=(hl == 0),
                                             stop=(hl == HPC - 1))
                        oo = epool.tile([128, 512], F16, tag="oo", bufs=4)
                        if nb % 2 == 0:
                            nc.vector.tensor_copy(oo[:], ps[:])
                            nc.sync.dma_start(
                                out[ch * CS + m01 * 128:
                                    ch * CS + m01 * 128 + 128, nsl], oo[:])
                        else:
                            nc.scalar.copy(oo[:], ps[:])
                            nc.scalar.dma_start(
                                out[ch * CS + m01 * 128:
                                    ch * CS + m01 * 128 + 128, nsl], oo[:])

            # emission order: P2 half0 right after P1 (its collective
            # completed mid-P1), first two chunks, then P2 half1 (its
            # collective completes around P1 end), then the rest.
            p2_half(0)
            p3_chunk(0)
            p3_chunk(1)
            p2_half(1)
            for ch in range(2, NCH):
                p3_chunk(ch)

            if debug and DEBUG_LVL >= 3:
                nc.sync.dma_start(
                    dbg("dbg_rf", [128, NCH * 2 * HPC]),
                    rf[:].rearrange("p a b c -> p (a b c)"))
                nc.sync.dma_start(
                    dbg("dbg_vf", [128, NCH * 2 * HPC]),
                    vf[:].rearrange("p a b c -> p (a b c)"))
                nc.sync.dma_start(
                    dbg("dbg_eS", [128, NCH * HPC]),
                    eS[:].rearrange("p a b -> p (a b)"))
            if debug and DEBUG_LVL >= 2:
                nc.sync.dma_start(
                    dbg("dbg_rsn", [128, NBLK * 3]),
                    rsn[:].rearrange("p a b -> p (a b)"))
                nc.sync.dma_start(
                    dbg("dbg_gtd", [128, NBLK * HPC]),
                    gtd[:].bitcast(F32).rearrange("p a b -> p (a b)"))

    nc.compile()
    return nc


def _prep_inputs(x, c, Wq, Wk, Wv, Wg, Wgt, Wo):
    """Build the 8 per-core input maps (host-side sharding / layout)."""
    consts = np.ascontiguousarray(_consts_np())
    c16 = np.concatenate(
        [np.eye(128, dtype=np.float16), np.ones((128, 1), np.float16)], axis=1)
    in_maps = []
    xTs = [np.ascontiguousarray(x[b].T.astype(np.float16)) for b in range(B)]
    xc = x + c
    xcTs = [np.ascontiguousarray(xc[b].T.astype(np.float16)) for b in range(B)]
    for core in range(NCORE):
        b, g = core // 4, core % 4
        cols = slice(g * PCOLS, (g + 1) * PCOLS)
        in_maps.append({
            "xT": xTs[b],
            "xcT": np.ascontiguousarray(xcTs[b][cols, :]),
            "wq": np.ascontiguousarray(Wq[:, cols]).astype(np.float16),
            "wk": np.ascontiguousarray(Wk[:, cols]).astype(np.float16),
            "wv": np.ascontiguousarray(Wv[:, cols]).astype(np.float16),
            "wg": np.ascontiguousarray(Wg[:, cols]).astype(np.float16),
            "wgt": np.ascontiguousarray(Wgt[cols, :]).astype(np.float16),
            "wo": np.ascontiguousarray(Wo[cols, :]).astype(np.float16),
            "consts": consts,
            "c16": c16,
        })
    return in_maps


def kernel(x, c, Wq, Wk, Wv, Wg, Wgt, Wo, _want_results=False):
    key = "nc_dbg" if DEBUG else "nc"
    if key not in _cache:
        _cache[key] = build(debug=DEBUG)
    nc = _cache[key]
    in_maps = _prep_inputs(np.asarray(x, np.float32), np.asarray(c, np.float32),
                           np.asarray(Wq, np.float32), np.asarray(Wk, np.float32),
                           np.asarray(Wv, np.float32), np.asarray(Wg, np.float32),
                           np.asarray(Wgt, np.float32), np.asarray(Wo, np.float32))
    res = bass_utils.run_bass_kernel_spmd(
        nc, in_maps, core_ids=list(range(NCORE)), trace=TRACE)
    out = np.zeros((B, T, DIM), np.float32)
    for core in range(NCORE):
        out[core // 4] += res.results[core]["out"].astype(np.float32)
    if _want_results:
        return out, res
    return out


# revision 38
# speedup vs baseline: 1.4617x; 1.0210x over previous
"""GateRetention Trainium2 kernel (Bass/Tile), 8-core tensor-parallel.

Sharding: core grid (batch b = core//4, head-group g = core%4); each core owns
4 heads (512 cols of the q/k/v/g projections, 512 rows of Wo) of one batch.
RMS-norm statistics are AllReduced across each batch's 4 cores; gate logits
are contraction-sharded (host pre-adds x+c, each core contracts a 512-row
quarter for all 16 heads) and ReduceScattered so each core gets its 4 heads.
Collectives are split per half-T and issued mid-P1 so they overlap compute.
Out-proj partials are summed on the host (row-parallel TP gather).

Precision: all projections and retention in fp16 with fp32 PSUM accumulation;
a 2^-2 exponent shift on vfac keeps decayed v tiles in fp16 range.  The
rowfac (per-token decay * q-norm * scale) is folded into the subln norm
scale f = rf * rsqrt(rf^2 * sumsq/HD + eps), so the raw retention output is
normalized+scaled in one pass (exactly equal to norm(rf*o_raw)).

kernel(**inputs) takes the FULL inputs from reference.setup_inputs() and
returns the FULL [B, T, DIM] fp32 output.
"""
import os
import sys

sys.path.insert(0, "/opt/trn_rl_repo")

import numpy as np

import concourse.bass as bass
import concourse.bacc as bacc
import concourse.tile as tile
import concourse.mybir as mybir
from concourse import bass_utils

F32 = mybir.dt.float32
F32R = mybir.dt.float32r
F16 = mybir.dt.float16
AX = mybir.AxisListType
ALU = mybir.AluOpType
ACTF = mybir.ActivationFunctionType

B, T, DIM = 2, 4096, 2048
H, HD = 16, 128
CS = 256
NCH = T // CS              # 16 chunks
EPS = 1e-5
GLN = 16.0
SCALE = HD ** -0.5
NCORE = 8
HPC = 4                    # heads per core
PCOLS = HPC * HD           # 512 cols per core
NBLK = T // 128            # 32 token blocks of 128
NT = T // 512              # 8 token n-tiles
TH = T // 2                # tokens per collective half
VSH = 2.0 ** -2            # fp16 range shift on vv; inverse folded into rowfac

DEBUG_LVL = int(os.environ.get("GR_DEBUG", "0"))
DEBUG = bool(DEBUG_LVL)
TRACE = bool(int(os.environ.get("GR_TRACE", "0")))
PET = not bool(int(os.environ.get("GR_XPT", "0")))  # PE transpose for o_n
NEWTON_ITERS = int(os.environ.get("GR_NEWTON", "1"))
KNPE = bool(int(os.environ.get("GR_KNPE", "0")))    # PE-transpose fallback for kn

_cache = {}


def _consts_np():
    """[128, 904] fp32: identity | Lm | Om | Um | ones8 | Lc | M0."""
    ident = np.eye(128, dtype=np.float32)
    jj, ii = np.meshgrid(np.arange(128), np.arange(128), indexing="ij")
    Lm = np.where(jj <= ii, -1.0 / GLN, 0.0).astype(np.float32)
    Om = np.full((128, 128), -1.0 / GLN, np.float32)
    Um = np.where(jj <= ii, 1.0, 0.0).astype(np.float32)
    ones = np.ones((128, 8), np.float32)
    ones[:, 1] = EPS                       # col 513: eps bias for Ln
    # Lc: b_i - b_mid for block0 = +1/GLN * sum_{j>i} sp_j
    Lc = np.where(jj > ii, 1.0 / GLN, 0.0).astype(np.float32)
    M0 = np.concatenate([Um, np.ones((128, 128), np.float32)], axis=1)
    return np.concatenate([ident, Lm, Om, Um, ones, Lc, M0], axis=1)


def build(debug=False):
    nc = bacc.Bacc("TRN2", target_bir_lowering=False, debug=False,
                   enable_asserts=False, num_devices=NCORE)

    # ---------------- I/O ----------------
    xT = nc.dram_tensor("xT", [DIM, T], F16, kind="ExternalInput").ap()
    # (x+c)^T row-quarter for this core's contraction shard of the gate logits
    xcT = nc.dram_tensor("xcT", [PCOLS, T], F16, kind="ExternalInput").ap()
    wq = nc.dram_tensor("wq", [DIM, PCOLS], F16, kind="ExternalInput").ap()
    wk = nc.dram_tensor("wk", [DIM, PCOLS], F16, kind="ExternalInput").ap()
    wv = nc.dram_tensor("wv", [DIM, PCOLS], F16, kind="ExternalInput").ap()
    wg = nc.dram_tensor("wg", [DIM, PCOLS], F16, kind="ExternalInput").ap()
    wgt = nc.dram_tensor("wgt", [PCOLS, H], F16, kind="ExternalInput").ap()
    wo = nc.dram_tensor("wo", [PCOLS, DIM], F16, kind="ExternalInput").ap()
    consts = nc.dram_tensor("consts", [128, 904], F32R, kind="ExternalInput").ap()
    c16 = nc.dram_tensor("c16", [128, 129], F16, kind="ExternalInput").ap()
    out = nc.dram_tensor("out", [T, DIM], F16, kind="ExternalOutput").ap()

    def dbg(name, shape, dtype=F32):
        return nc.dram_tensor(name, shape, dtype, kind="ExternalOutput").ap()

    with tile.TileContext(nc) as tc:
        with (
            tc.tile_pool(name="const", bufs=1) as cpool,
            tc.tile_pool(name="wts", bufs=1) as wpool,
            tc.tile_pool(name="xstream", bufs=4) as xpool,
            tc.tile_pool(name="cstream", bufs=2) as ctpool,
            tc.tile_pool(name="evac", bufs=4) as epool,
            tc.tile_pool(name="persist", bufs=1) as ppool,
            tc.tile_pool(name="small", bufs=2) as spool,
            tc.tile_pool(name="ret", bufs=2) as rpool,
            tc.tile_pool(name="ps", bufs=1, space="PSUM") as psp,
            tc.tile_pool(name="dram", bufs=1, space="DRAM") as dpool,
        ):
            def ps_big():
                return psp.tile([128, 512], F32, tag="big", bufs=2, name="psbig")

            def ps_small(shape=None, dtype=F32):
                return psp.tile(shape or [128, 256], dtype, tag="small", bufs=2,
                                name="pssmall")

            def ps_oraw():
                return psp.tile([128, HPC, HD], F32, tag="oraw", bufs=2,
                                name="psoraw")

            def ps_trp():
                return psp.tile([128, 128], F16, tag="trp", bufs=2,
                                name="pstrp")

            # ---------------- constants ----------------
            cst = cpool.tile([128, 904], F32R, tag="consts")
            nc.sync.dma_start(cst[:], consts)
            ident32 = cst[:, 0:128].bitcast(F32)
            Lm = cst[:, 128:256]
            Om = cst[:, 256:384]
            Um32 = cst[:, 384:512].bitcast(F32)
            epsb = cst[:, 513:514].bitcast(F32)
            Lc = cst[:, 520:648]
            M0 = cst[:, 648:904].bitcast(F32)
            aux16 = cpool.tile([128, 129], F16, tag="i16")
            nc.sync.dma_start(aux16[:], c16)
            i16 = aux16[:, 0:128]
            ones16 = aux16[:, 128:129]

            # ---------------- DRAM scratch (fp16) ----------------
            if debug:
                qT_s = dbg("dbg_qT", [PCOLS, T], F16)
                kT_s = dbg("dbg_kT", [PCOLS, T], F16)
                vN_s = dbg("dbg_vN", [T, PCOLS], F16)
                gT_s = dbg("dbg_gT", [PCOLS, T], F16)
            else:
                qT_s = nc.dram_tensor("qT_s", [PCOLS, T], F16,
                                      kind="Internal").ap()
                kT_s = nc.dram_tensor("kT_s", [PCOLS, T], F16,
                                      kind="Internal").ap()
                vN_s = nc.dram_tensor("vN_s", [T, PCOLS], F16,
                                      kind="Internal").ap()
                gT_s = nc.dram_tensor("gT_s", [PCOLS, T], F16,
                                      kind="Internal").ap()
            ss_in = [nc.dram_tensor(f"ss_in{h}", [3, TH], F32,
                                    kind="Internal").ap() for h in range(2)]
            ss_out = [nc.dram_tensor(f"ss_out{h}", [3, TH], F32,
                                     kind="Internal").ap() for h in range(2)]
            gt_in = [nc.dram_tensor(f"gt_in{h}", [H, TH], F32,
                                    kind="Internal").ap() for h in range(2)]
            gt_out = [nc.dram_tensor(f"gt_out{h}", [HPC, TH], F32,
                                     kind="Internal").ap() for h in range(2)]

            # =========================================================
            # P1: single fused pass: q,k,g (T-layout), v (natural),
            # gate-logit partials; all fp16 math, fp32 PSUM.
            # Collectives per half-T, issued mid-stream so they overlap.
            # =========================================================
            vss = ppool.tile([128, NBLK], F32, tag="vss")

            def load_w(wdram, tag):
                wt = wpool.tile([128, 16, 512], F16, tag=tag)
                nc.sync.dma_start(
                    wt[:], wdram.rearrange("(kt p) m -> p kt m", p=128))
                return wt

            wq_sb = load_w(wq, "w0")
            wk_sb = load_w(wk, "w1")
            wv_sb = load_w(wv, "w2")
            wg_sb = load_w(wg, "w3")
            wgt_sb = wpool.tile([128, HPC, H], F16, tag="wgt")
            nc.sync.dma_start(wgt_sb[:],
                              wgt.rearrange("(kt p) m -> p kt m", p=128))

            I32 = mybir.dt.int32

            def newton_rsqrt(out, din, tmp, iters=2):
                """out = rsqrt(din) on the vector engine only (no ACT table
                loads): bit-trick seed + Newton iterations."""
                oi = out.bitcast(I32)
                di = din.bitcast(I32)
                nc.vector.tensor_scalar(oi, di, 1, None,
                                        ALU.logical_shift_right)
                nc.vector.tensor_scalar(oi, oi, -1, 0x5f3759df, ALU.mult,
                                        ALU.add)
                for _ in range(iters):
                    nc.vector.tensor_tensor(tmp, out, out, ALU.mult)
                    nc.vector.tensor_tensor(tmp, tmp, din, ALU.mult)
                    nc.vector.tensor_scalar(tmp, tmp, -0.5, 1.5, ALU.mult,
                                            ALU.add)
                    nc.vector.tensor_tensor(out, out, tmp, ALU.mult)

            def tproj_mms(ps, wt, xth, m):
                for k in range(16):
                    nc.tensor.matmul(
                        ps[:], wt[:, k, m * 128:(m + 1) * 128],
                        xth[k // 8][:, k % 8, :], start=(k == 0), stop=(k == 15))

            for half in range(2):
                for n in range(half * 4, half * 4 + 4):
                    tok = slice(n * 512, (n + 1) * 512)
                    ltok = slice(n * 512 - half * TH, (n + 1) * 512 - half * TH)
                    xth = []
                    for h2 in range(2):
                        xt = xpool.tile([128, 8, 512], F16, tag="xt")
                        nc.sync.dma_start(
                            xt[:], xT[h2 * 1024:(h2 + 1) * 1024, tok].rearrange(
                                "(kt p) m -> p kt m", p=128))
                        xth.append(xt)
                    # -- q, k: T-layout; squares kept for deferred sumsq
                    sqs = {0: [], 1: []}
                    for pi, (wsb, sdram) in enumerate(((wq_sb, qT_s),
                                                       (wk_sb, kT_s))):
                        for m in range(4):
                            ps = ps_big()
                            tproj_mms(ps, wsb, xth, m)
                            ev = epool.tile([128, 512], F16, tag="ev")
                            sqt = epool.tile([128, 512], F16, tag="sq",
                                             bufs=10)
                            if m % 2 == 0:
                                nc.vector.tensor_copy(ev[:], ps[:])
                                nc.scalar.activation(sqt[:], ps[:],
                                                     ACTF.Square)
                                nc.sync.dma_start(
                                    sdram[m * 128:(m + 1) * 128, tok], ev[:])
                            else:
                                nc.scalar.copy(ev[:], ps[:])
                                nc.vector.tensor_tensor(sqt[:], ev[:], ev[:],
                                                        ALU.mult)
                                nc.scalar.dma_start(
                                    sdram[m * 128:(m + 1) * 128, tok], ev[:])
                            sqs[pi].append(sqt)
                    # -- g: silu, T-layout
                    for m in range(4):
                        ps = ps_big()
                        tproj_mms(ps, wg_sb, xth, m)
                        ev = epool.tile([128, 512], F16, tag="ev")
                        nc.scalar.activation(ev[:], ps[:], ACTF.Silu)
                        nc.scalar.dma_start(gT_s[m * 128:(m + 1) * 128, tok],
                                            ev[:])
                    # -- v: natural layout + accumulated sumsq
                    for mt in range(4):
                        msl = slice(mt * 128, (mt + 1) * 128)
                        ps = ps_big()
                        for k in range(16):
                            nc.tensor.matmul(
                                ps[:], xth[k // 8][:, k % 8, msl],
                                wv_sb[:, k, :], start=(k == 0), stop=(k == 15))
                        ev = epool.tile([128, 512], F16, tag="ev")
                        sqt = epool.tile([128, 512], F16, tag="vsq", bufs=2)
                        nc.vector.tensor_copy(ev[:], ps[:])
                        nc.scalar.activation(
                            sqt[:], ps[:], ACTF.Square,
                            accum_out=vss[:, n * 4 + mt:n * 4 + mt + 1])
                        nc.sync.dma_start(
                            vN_s[n * 512 + mt * 128:n * 512 + (mt + 1) * 128,
                                 :], ev[:])
                    # -- gate logits: contraction-sharded over (x+c) quarter
                    ct = ctpool.tile([128, HPC, 512], F16, tag="ct")
                    nc.sync.dma_start(
                        ct[:], xcT[:, tok].rearrange("(kt p) m -> p kt m",
                                                     p=128))
                    gtps = ps_small([H, 512])
                    for kk in range(HPC):
                        nc.tensor.matmul(gtps[:H, :], wgt_sb[:, kk, :],
                                         ct[:, kk, :], start=(kk == 0),
                                         stop=(kk == HPC - 1))
                    gstg = spool.tile([H, 512], F32, tag="gstg", bufs=2)
                    nc.vector.tensor_copy(gstg[:], gtps[:H, :])
                    nc.scalar.dma_start(gt_in[half][:, ltok], gstg[:])
                    # -- deferred sumsq matmuls (inputs long since evacuated)
                    for pi in range(2):
                        ssps = ps_small([1, 512])
                        for m in range(4):
                            nc.tensor.matmul(ssps[:1, :], ones16,
                                             sqs[pi][m][:], start=(m == 0),
                                             stop=(m == 3))
                        ssev = spool.tile([1, 512], F32, tag="ssev", bufs=2)
                        nc.vector.tensor_copy(ssev[:], ssps[:1, :])
                        nc.sync.dma_start(ss_in[half][pi:pi + 1, ltok],
                                          ssev[:])
                # v sumsq for this half: transpose [128,16] -> row 2
                vssT = ps_small([128, 128])
                nc.tensor.matmul(vssT[:16, :],
                                 vss[:, half * 16:(half + 1) * 16], ident32,
                                 is_transpose=True)
                vssev = spool.tile([16, 128], F32, tag="vssev", bufs=2)
                nc.vector.tensor_copy(vssev[:], vssT[:16, :])
                nc.sync.dma_start(
                    ss_in[half][2:3, :].rearrange("a (b c) -> (a b) c", c=128),
                    vssev[:])
                # collectives for this half (overlap with the next half's P1)
                nc.gpsimd.collective_compute(
                    "AllReduce", ALU.add,
                    replica_groups=[[0, 1, 2, 3], [4, 5, 6, 7]],
                    ins=[ss_in[half].opt()], outs=[ss_out[half].opt()],
                )
                nc.gpsimd.collective_compute(
                    "ReduceScatter", ALU.add,
                    replica_groups=[[0, 1, 2, 3], [4, 5, 6, 7]],
                    ins=[gt_in[half].opt()], outs=[gt_out[half].opt()],
                )

            # =========================================================
            # P2 (per half): norm scales + gate decays
            # =========================================================
            ssgt = ppool.tile([128, NBLK, 7], F32, tag="ssgt")
            rsn = ppool.tile([128, NBLK, 3], F32, tag="rsn")
            skv = ppool.tile([128, NBLK], F32, tag="skv")
            gtd = ppool.tile([128, NBLK, HPC], F32R, tag="gtd")
            gtn = ppool.tile([128, NBLK, HPC], F32, tag="gtn")
            rf = ppool.tile([128, NCH, 2, HPC], F32, tag="rf")      # rowfac
            vf = ppool.tile([128, NCH, 2, HPC], F32, tag="vf")      # vfac
            eS = ppool.tile([128, NCH, HPC], F32, tag="eS")
            rf2 = ppool.tile([128, NCH, 2, HPC], F32, tag="rf2")

            def es_part(ch):
                # eS[ch] couples chunk ch and ch+1 (blocks 2ch+1, 2ch+2)
                b1 = 2 * ch + 1
                pt = ps_small([128, HPC])
                nc.tensor.matmul(pt[:], Om, gtd[:, b1, :], start=True,
                                 stop=False)
                nc.tensor.matmul(pt[:], Om, gtd[:, b1 + 1, :], start=False,
                                 stop=True)
                nc.scalar.activation(eS[:, ch, :], pt[:], ACTF.Exp)

            def p2_half(half):
                hb = slice(half * 16, (half + 1) * 16)
                for nn_ in range(4):
                    ltok = slice(nn_ * 512, (nn_ + 1) * 512)
                    srt = spool.tile([7, 512], F32, tag="srt", bufs=2)
                    nc.sync.dma_start(srt[0:3, :], ss_out[half][:, ltok])
                    nc.sync.dma_start(srt[3:7, :], gt_out[half][:, ltok])
                    for j in range(4):
                        tp = ps_small([128, 8])
                        nc.tensor.matmul(tp[:, :7],
                                         srt[:, j * 128:(j + 1) * 128],
                                         ident32[:7, :7], is_transpose=True)
                        nc.vector.tensor_copy(
                            ssgt[:, half * 16 + nn_ * 4 + j, :], tp[:, :7])
                rsw = spool.tile([128, 16, 3], F32, tag="rsw", bufs=2)
                rst = spool.tile([128, 16, 3], F32, tag="rst", bufs=2)
                nc.vector.tensor_scalar(rsw[:], ssgt[:, hb, 0:3],
                                        1.0 / DIM, EPS, ALU.mult, ALU.add)
                newton_rsqrt(rsn[:, hb], rsw[:], rst[:])
                nc.vector.tensor_mul(skv[:, hb], rsn[:, hb, 1], rsn[:, hb, 2])
                nc.scalar.activation(gtn[:, hb], ssgt[:, hb, 3:7], ACTF.Exp,
                                     scale=-1.0)
                nc.scalar.activation(gtd[:, hb], gtn[:, hb], ACTF.Ln, bias=1.0)
                if half == 1:
                    es_part(7)  # needs block 16 (half 1), deferred to here
                for ch in range(half * 8, half * 8 + 8):
                    b0, b1 = 2 * ch, 2 * ch + 1
                    p0 = ps_small([128, HPC])
                    nc.tensor.matmul(p0[:], Lc, gtd[:, b0, :], start=True,
                                     stop=True)
                    p1 = ps_small([128, HPC])
                    nc.tensor.matmul(p1[:], Lm, gtd[:, b1, :], start=True,
                                     stop=True)
                    if ch < NCH - 1 and ch != 7:
                        es_part(ch)
                    for blk01, bps in ((0, p0), (1, p1)):
                        blk = 2 * ch + blk01
                        # rowfac = exp(b') * sq * scale / VSH
                        nc.scalar.activation(rf[:, ch, blk01, :], bps[:],
                                             ACTF.Exp)
                        nc.vector.tensor_scalar(
                            rf[:, ch, blk01, :], rf[:, ch, blk01, :],
                            rsn[:, blk, 0:1], SCALE / VSH, ALU.mult, ALU.mult)
                        # vfac = exp(-b') * sk * sv * VSH
                        nc.scalar.activation(vf[:, ch, blk01, :], bps[:],
                                             ACTF.Exp, scale=-1.0)
                        nc.vector.tensor_scalar(
                            vf[:, ch, blk01, :], vf[:, ch, blk01, :],
                            skv[:, blk:blk + 1], VSH, ALU.mult, ALU.mult)
                # rf2 = rf^2 / HD (for the fused norm scale)
                nc.vector.scalar_tensor_tensor(
                    rf2[:, half * 8:(half + 1) * 8],
                    rf[:, half * 8:(half + 1) * 8],
                    1.0 / HD, rf[:, half * 8:(half + 1) * 8],
                    op0=ALU.mult, op1=ALU.mult)

            # =========================================================
            # P3: retention + fused norm/gate + out-proj, per chunk
            # =========================================================
            if int(os.environ.get("GR_BARRIER", "0")):
                tc.prologue_barrier()
            wo_sb = wpool.tile([128, HPC, DIM], F16, tag="wo")
            nc.sync.dma_start(wo_sb[:], wo.rearrange("(h p) m -> p h m", p=128))

            S_box = [None]

            def p3_chunk(ch):
                S_prev = S_box[0]
                tok = slice(ch * CS, (ch + 1) * CS)
                qc = rpool.tile([128, HPC, CS], F16, tag="qc")
                kc = rpool.tile([128, HPC, CS], F16, tag="kc")
                for t_, s_ in ((qc, qT_s), (kc, kT_s)):
                    nc.sync.dma_start(
                        t_[:], s_[:, tok].rearrange("(h p) m -> p h m", p=128))
                # k natural layout: one batched XBAR transpose per token block
                kn = []
                if ch < NCH - 1:
                    if KNPE:
                        for hl in range(HPC):
                            for ci in range(2):
                                tpk = ps_trp()
                                nc.tensor.transpose(
                                    tpk[:], kc[:, hl,
                                               ci * 128:ci * 128 + 128], i16)
                                knt = rpool.tile([128, 128], F16,
                                                 tag=f"kn{hl * 2 + ci}")
                                if (hl + ci) % 2 == 0:
                                    nc.scalar.copy(knt[:], tpk[:])
                                else:
                                    nc.vector.tensor_copy(knt[:], tpk[:])
                                kn.append(knt)
                    else:
                        for ci in range(2):
                            bt = slice(ch * CS + ci * 128,
                                       ch * CS + ci * 128 + 128)
                            knb = rpool.tile([128, PCOLS], F16,
                                             tag=f"knb{ci}")
                            nc.sync.dma_start_transpose(
                                knb[:], kT_s[:, bt])
                            kn.append(knb)
                vcn = []
                for ci in range(2):
                    bt = slice(ch * CS + ci * 128, ch * CS + ci * 128 + 128)
                    vt = rpool.tile([128, PCOLS], F16, tag="vcn", bufs=4)
                    nc.gpsimd.dma_start(vt[:], vN_s[bt, :])
                    vcn.append(vt)
                sg = rpool.tile([128, HPC, CS], F16, tag="sg")
                nc.scalar.dma_start(
                    sg[:], gT_s[:, tok].rearrange("(h p) m -> p h m", p=128))
                # vv = v * vfac
                vvt = rpool.tile([128, 2, HPC, HD], F16, tag="vvt")
                for ci in range(2):
                    for hl in range(HPC):
                        nc.vector.tensor_scalar(
                            vvt[:, ci, hl, :],
                            vcn[ci][:, hl * 128:(hl + 1) * 128],
                            vf[:, ch, ci, hl:hl + 1], None, ALU.mult)
                # state update FIRST: the S recurrence is the only true
                # cross-chunk serial chain, so emit it ahead of everything
                # else; o_raw below still uses the previous chunk's state.
                if ch < NCH - 1:
                    sps = ps_small([128, HPC, HD])
                    for hl in range(HPC):
                        hsl = slice(hl * 128, (hl + 1) * 128)
                        kn0 = kn[hl * 2][:] if KNPE else kn[0][:, hsl]
                        kn1 = kn[hl * 2 + 1][:] if KNPE else kn[1][:, hsl]
                        nc.tensor.matmul(sps[:, hl, :], kn0,
                                         vvt[:, 0, hl, :], start=True,
                                         stop=False)
                        nc.tensor.matmul(sps[:, hl, :], kn1,
                                         vvt[:, 1, hl, :], start=False,
                                         stop=True)
                    eSb = eS[:, ch, :].unsqueeze(2).to_broadcast(
                        [128, HPC, HD])
                    S_cur = rpool.tile([128, HPC, HD], F16, tag="S")
                    if ch > 0:
                        stmp = rpool.tile([128, HPC, HD], F32, tag="stmp")
                        nc.vector.tensor_tensor(stmp[:], sps[:], S_prev[:],
                                                ALU.add)
                        nc.vector.tensor_tensor(S_cur[:], stmp[:], eSb,
                                                ALU.mult)
                    else:
                        nc.vector.tensor_tensor(S_cur[:], sps[:], eSb,
                                                ALU.mult)
                    S_box[0] = S_cur
                # AT (masked): rows tj, cols ti
                at0s, at1s = [], []
                for hl in range(HPC):
                    at0ps = ps_small([128, 256])
                    nc.tensor.matmul(at0ps[:], kc[:, hl, 0:128], qc[:, hl, :],
                                     start=True, stop=True)
                    at0 = rpool.tile([128, CS], F16, tag="at0", bufs=4)
                    nc.vector.scalar_tensor_tensor(
                        at0[:], at0ps[:], 1.0, M0, op0=ALU.mult, op1=ALU.mult)
                    at0s.append(at0)
                    at1ps = ps_small([128, 128])
                    nc.tensor.matmul(at1ps[:], kc[:, hl, 128:256],
                                     qc[:, hl, 128:256], start=True, stop=True)
                    at1 = rpool.tile([128, 128], F16, tag="at1s", bufs=4)
                    nc.vector.scalar_tensor_tensor(
                        at1[:], at1ps[:], 1.0, Um32, op0=ALU.mult,
                        op1=ALU.mult)
                    at1s.append(at1)
                # o_raw = intra + inter; one PSUM tile per token half-block
                orp = []
                for ci in range(2):
                    orps = ps_oraw()
                    for hl in range(HPC):
                        reg = orps[:, hl, :]
                        mms = [(at0s[hl][:, ci * 128:ci * 128 + 128],
                                vvt[:, 0, hl, :])]
                        if ci == 1:
                            mms.append((at1s[hl][:], vvt[:, 1, hl, :]))
                        if ch > 0:
                            mms.append((qc[:, hl, ci * 128:ci * 128 + 128],
                                        S_prev[:, hl, :]))
                        for i, (lh, rh) in enumerate(mms):
                            nc.tensor.matmul(reg, lh, rh, start=(i == 0),
                                             stop=(i == len(mms) - 1))
                    orp.append(orps)
                # fused subln norm + rowfac: f = rf*rsqrt(rf^2*ss/HD + eps)
                ssum = rpool.tile([128, 2 * HPC], F32, tag="ssum")
                for ci in range(2):
                    osq = rpool.tile([128, HPC, HD], F32, tag="osq", bufs=2)
                    nc.scalar.activation(osq[:], orp[ci][:], ACTF.Square)
                    nc.vector.tensor_reduce(
                        ssum[:, ci * HPC:(ci + 1) * HPC], osq[:], AX.X,
                        ALU.add)
                rfv = rf[:, ch].rearrange("p a b -> p (a b)")
                rf2v = rf2[:, ch].rearrange("p a b -> p (a b)")
                dd = rpool.tile([128, 2 * HPC], F32, tag="dd")
                nc.vector.tensor_tensor(dd[:], rf2v, ssum[:], ALU.mult)
                nc.vector.tensor_scalar(dd[:], dd[:], EPS, None, ALU.add)
                yy = rpool.tile([128, 2 * HPC], F32, tag="yy")
                yt = rpool.tile([128, 2 * HPC], F32, tag="yt")
                newton_rsqrt(yy[:], dd[:], yt[:], iters=NEWTON_ITERS)
                ff = rpool.tile([128, 2 * HPC], F32, tag="ff")
                nc.vector.tensor_tensor(ff[:], rfv, yy[:], ALU.mult)
                o_n = rpool.tile([128, 2 * HPC, HD], F16, tag="o_n")
                for ci in range(2):
                    hsl = slice(ci * HPC, (ci + 1) * HPC)
                    nc.vector.tensor_tensor(
                        o_n[:, hsl, :], orp[ci][:],
                        ff[:, hsl].unsqueeze(2).to_broadcast([128, HPC, HD]),
                        ALU.mult)
                # transpose to [chan, tok] + gate (idx = ci*HPC + hl)
                go = []
                for ci in range(2):
                    for hl in range(HPC):
                        idx = ci * HPC + hl
                        got = rpool.tile([128, 128], F16, tag=f"go{idx}")
                        if PET:
                            trp = ps_trp()
                            nc.tensor.transpose(trp[:], o_n[:, idx, :], i16)
                            nc.vector.tensor_mul(
                                got[:], trp[:],
                                sg[:, hl, ci * 128:ci * 128 + 128])
                        else:
                            tro = rpool.tile([128, 128], F16, tag=f"tr{idx}")
                            nc.sync.dma_start_transpose(
                                tro[:], o_n[:, idx, :])
                            nc.vector.tensor_mul(
                                got[:], tro[:],
                                sg[:, hl, ci * 128:ci * 128 + 128])
                        go.append(got)
                # out-proj for this chunk's two token tiles
                for m01 in range(2):
                    for nb in range(DIM // 512):
                        ps = ps_big()
                        nsl = slice(nb * 512, (nb + 1) * 512)
                        for hl in range(HPC):
                            nc.tensor.matmul(ps[:], go[m01 * HPC + hl][:],
                                             wo_sb[:, hl, nsl],
                                             start=(hl == 0),
                                             stop=(hl == HPC - 1))
                        oo = epool.tile([128, 512], F16, tag="oo", bufs=4)
                        if nb == 0:
                            nc.vector.tensor_copy(oo[:], ps[:])
                            nc.sync.dma_start(
                                out[ch * CS + m01 * 128:
                                    ch * CS + m01 * 128 + 128, nsl], oo[:])
                        else:
                            nc.scalar.copy(oo[:], ps[:])
                            eng = nc.scalar if nb % 2 else nc.gpsimd
                            eng.dma_start(
                                out[ch * CS + m01 * 128:
                                    ch * CS + m01 * 128 + 128, nsl], oo[:])

            # emission order: P2 half0 right after P1 (its collective
            # completed mid-P1), first two chunks, then P2 half1 (its
            # collective completes around P1 end), then the rest.
            p2_half(0)
            p3_chunk(0)
            p3_chunk(1)
            p2_half(1)
            for ch in range(2, NCH):
                p3_chunk(ch)

            if debug and DEBUG_LVL >= 3:
                nc.sync.dma_start(
                    dbg("dbg_rf", [128, NCH * 2 * HPC]),
                    rf[:].rearrange("p a b c -> p (a b c)"))
                nc.sync.dma_start(
                    dbg("dbg_vf", [128, NCH * 2 * HPC]),
                    vf[:].rearrange("p a b c -> p (a b c)"))
                nc.sync.dma_start(
                    dbg("dbg_eS", [128, NCH * HPC]),
                    eS[:].rearrange("p a b -> p (a b)"))
            if debug and DEBUG_LVL >= 2:
                nc.sync.dma_start(
                    dbg("dbg_rsn", [128, NBLK * 3]),
                    rsn[:].rearrange("p a b -> p (a b)"))
                nc.sync.dma_start(
                    dbg("dbg_gtd", [128, NBLK * HPC]),
                    gtd[:].bitcast(F32).rearrange("p a b -> p (a b)"))

    nc.compile()
    return nc


def _prep_inputs(x, c, Wq, Wk, Wv, Wg, Wgt, Wo):
    """Build the 8 per-core input maps (host-side sharding / layout)."""
    consts = np.ascontiguousarray(_consts_np())
    c16 = np.concatenate(
        [np.eye(128, dtype=np.float16), np.ones((128, 1), np.float16)], axis=1)
    in_maps = []
    xTs = [np.ascontiguousarray(x[b].T.astype(np.float16)) for b in range(B)]
    xc = x + c
    xcTs = [np.ascontiguousarray(xc[b].T.astype(np.float16)) for b in range(B)]
    for core in range(NCORE):
        b, g = core // 4, core % 4
        cols = slice(g * PCOLS, (g + 1) * PCOLS)
        in_maps.append({
            "xT": xTs[b],
            "xcT": np.ascontiguousarray(xcTs[b][cols, :]),
            "wq": np.ascontiguousarray(Wq[:, cols]).astype(np.float16),
            "wk": np.ascontiguousarray(Wk[:, cols]).astype(np.float16),
            "wv": np.ascontiguousarray(Wv[:, cols]).astype(np.float16),
            "wg": np.ascontiguousarray(Wg[:, cols]).astype(np.float16),
            "wgt": np.ascontiguousarray(Wgt[cols, :]).astype(np.float16),
            "wo": np.ascontiguousarray(Wo[cols, :]).astype(np.float16),
            "consts": consts,
            "c16": c16,
        })
    return in_maps


def kernel(x, c, Wq, Wk, Wv, Wg, Wgt, Wo, _want_results=False):
    key = "nc_dbg" if DEBUG else "nc"
    if key not in _cache:
        _cache[key] = build(debug=DEBUG)
    nc = _cache[key]
    in_maps = _prep_inputs(np.asarray(x, np.float32), np.asarray(c, np.float32),
                           np.asarray(Wq, np.float32), np.asarray(Wk, np.float32),
                           np.asarray(Wv, np.float32), np.asarray(Wg, np.float32),
                           np.asarray(Wgt, np.float32), np.asarray(Wo, np.float32))
    res = bass_utils.run_bass_kernel_spmd(
        nc, in_maps, core_ids=list(range(NCORE)), trace=TRACE)
    out = np.zeros((B, T, DIM), np.float32)
    for core in range(NCORE):
        out[core // 4] += res.results[core]["out"].astype(np.float32)
    if _want_results:
        return out, res
    return out
